# revision 19
# baseline (speedup 1.0000x reference)
"""Trainium2 Bass kernel for nn_BipartiteMessagePassingLayer.

Reference computation (per batch element, flattened over the batch):
    attr_msgs = flat_attrs[src_attr] @ W_a2o.T + b_a2o          # [E, 256]
    weighted  = attr_msgs * w[:, None]
    agg       = segsum(weighted, src_obj) / clip(segsum(w, src_obj), 1e-6)
    proj      = agg @ W_proj.T + b_proj
    out       = relu(concat([X, proj]) @ W_upd.T + b_upd)

Algebraic restructuring used here (exact up to float reassociation):
    Let W1 = W_upd[:, :256], W2 = W_upd[:, 256:]
    Let Wc = W2 @ W_proj @ W_a2o        (256 x 128)
    Let s_o = sum_e  (w_e / c_o) * flat_attrs[src_attr_e]   over edges of o,
        where c_o = max(segsum(w)_o, 1e-6)   (weights normalized host-side)
    out = relu( X @ W1.T + s @ Wc.T + r1_o * (W2@W_proj@b_a2o) + (W2@b_proj + b_upd) )
        with r1_o = segsum(w)_o / c_o  (0 or 1 for all practical inputs).

Sharding: data-parallel over batch. Core c owns objects of batch c; edges are
bucketed to the core owning their src_obj.  The full attr table (16000 x 128,
bf16) is replicated into SBUF of every core; per-edge attr rows are fetched
with the SBUF-source transposed dma_gather, weighted via a PE outer-product
broadcast of the edge weights + DVE multiply, and segment-summed with a
strided DVE tensor_reduce over host-built fixed-capacity degree buckets.
Dense work (X @ W1.T + s @ Wc.T, relu) runs on the PE in bf16.
"""

import os

os.environ.setdefault("JAX_PLATFORMS", "")

import numpy as np
import ml_dtypes

import concourse.bass as bass
import concourse.tile as tile
from concourse import bacc, mybir
from concourse.bass_utils import run_bass_kernel_spmd
from concourse.masks import make_identity

# ---------------------------------------------------------------- constants
B, N_OBJ, N_ATTR = 8, 10000, 2000
IN_DIM, OUT_DIM, ATTR_DIM = 256, 256, 128
E_TOT = 800000
NCORES = 8
NA_TOT = B * N_ATTR  # 16000 attr rows, gathered globally

CHUNK_TOK = 2048  # tokens per gather/mult chunk (psum: 2048 f32 = 4 banks)
R_BASE = [4, 8, 12, 16, 24, 32, 64]  # degree-bucket capacities

_BF16 = ml_dtypes.bfloat16

_compiled_cache: dict = {}


# ================================================================ host prep
def _bucket_structure(degs_per_core):
    """Pick a common (all-cores) bucket structure.

    Returns (r_list, n_per_bucket) with n_per_bucket counts that are >= the
    per-core needs, and sum(n) a multiple of 128."""
    max_deg = max(int(d.max()) for d in degs_per_core)
    r_list = list(R_BASE)
    while r_list[-1] < max_deg:
        r_list.append(r_list[-1] * 2)
    r_arr = np.asarray(r_list)
    counts = np.zeros(len(r_list), dtype=np.int64)
    for d in degs_per_core:
        b = np.searchsorted(r_arr, np.maximum(d, 1))
        counts = np.maximum(counts, np.bincount(b, minlength=len(r_list)))
    counts = counts + 16  # per-bucket slack
    n_slot = int(counts.sum())
    pad = (-n_slot) % 128
    counts[0] += pad  # pad N_SLOT to a multiple of 128 with tiny-bucket slots
    return r_list, [int(c) for c in counts]


def _chunk_plan(r_list, n_per_bucket):
    """Static chunk list shared by all cores.

    Each chunk: (tok0, ntok, nobj, R, slot0).  ntok is a multiple of 128,
    whole objects only (nobj*R <= ntok)."""
    chunks = []
    tok0 = 0
    slot0 = 0
    for R, n in zip(r_list, n_per_bucket):
        opc = CHUNK_TOK // R  # objects per full chunk
        i = 0
        while i < n:
            nobj = min(opc, n - i)
            ntok = -(-nobj * R // 128) * 128  # round up to 128
            chunks.append((tok0, ntok, nobj, R, slot0 + i))
            tok0 += ntok
            i += nobj
        slot0 += n
    t_total = tok0
    n_slot = slot0
    assert n_slot == sum(n_per_bucket)
    return chunks, t_total, n_slot


def _prepare(object_feats, attr_feats, edge_index, edge_weight,
             W_a2o, b_a2o, W_proj, b_proj, W_upd, b_upd):
    """All host-side sharding/packing. Returns (structure, in_maps, meta)."""
    src_obj = np.asarray(edge_index[0]).astype(np.int64)
    src_attr = np.asarray(edge_index[1]).astype(np.int64)
    w = np.asarray(edge_weight, dtype=np.float32)
    X = np.asarray(object_feats, dtype=np.float32)
    A = np.asarray(attr_feats, dtype=np.float32).reshape(NA_TOT, ATTR_DIM)

    core_of = src_obj // N_OBJ
    per_core = []
    degs = []
    for c in range(NCORES):
        m = core_of == c
        lobj = (src_obj[m] - c * N_OBJ).astype(np.int64)
        per_core.append((lobj, src_attr[m].astype(np.int64), w[m]))
        degs.append(np.bincount(lobj, minlength=N_OBJ))

    r_list, n_per_bucket = _bucket_structure(degs)
    chunks, T, n_slot = _chunk_plan(r_list, n_per_bucket)
    r_arr = np.asarray(r_list)

    # ---- weights (host factoring; exact in f64 then cast) ----
    W_upd64 = np.asarray(W_upd, dtype=np.float64)
    W1 = W_upd64[:, :IN_DIM]
    W2 = W_upd64[:, IN_DIM:]
    Wp = np.asarray(W_proj, dtype=np.float64)
    Wa = np.asarray(W_a2o, dtype=np.float64)
    Wc = W2 @ Wp @ Wa  # [256, 128]
    ba2 = W2 @ Wp @ np.asarray(b_a2o, dtype=np.float64)
    bconst = W2 @ np.asarray(b_proj, dtype=np.float64) + np.asarray(b_upd, np.float64)
    has_bias = bool(np.any(ba2 != 0.0) or np.any(bconst != 0.0))

    w1t_bf = np.ascontiguousarray(
        W1.T.reshape(2, 128, OUT_DIM).astype(_BF16))        # [2,128,256]
    wct_bf = np.ascontiguousarray(Wc.T.astype(_BF16))       # [128,256]
    bias_rhs = np.stack([ba2, bconst]).astype(_BF16)        # [2,256]

    # attr table rearranged so row i sits at partition i%128, rank i//128
    attr_re = np.ascontiguousarray(
        A.reshape(NA_TOT // 128, 128, ATTR_DIM).transpose(1, 0, 2)
        .reshape(128, (NA_TOT // 128) * ATTR_DIM).astype(_BF16))

    in_maps = []
    metas = []
    for c in range(NCORES):
        lobj, lattr, wv = per_core[c]
        deg = degs[c]
        ws = np.zeros(N_OBJ, dtype=np.float64)
        np.add.at(ws, lobj, wv.astype(np.float64))
        assert not np.any((ws > 0) & (ws <= 2e-6)), "r1 edge case hit"
        cdenom = np.maximum(ws, 1e-6)
        wprime = (wv / cdenom[lobj]).astype(np.float32)

        # slot assignment: bucket-major, real objects then dummies
        bucket_of = np.searchsorted(r_arr, np.maximum(deg, 1))
        perm = np.full(n_slot, -1, dtype=np.int64)  # slot -> local obj id
        slot_base = 0
        obj_slot = np.empty(N_OBJ, dtype=np.int64)
        for bi, n in enumerate(n_per_bucket):
            objs = np.nonzero(bucket_of == bi)[0]
            assert len(objs) <= n, (bi, len(objs), n)
            perm[slot_base:slot_base + len(objs)] = objs
            obj_slot[objs] = slot_base + np.arange(len(objs))
            slot_base += n

        # token stream: edges sorted by slot; per-object tokens padded to R
        order = np.argsort(obj_slot[lobj], kind="stable")
        s_lattr = lattr[order]
        s_wp = wprime[order]
        s_slot = obj_slot[lobj][order]

        tok_attr = np.zeros(T, dtype=np.int16)
        tok_w = np.zeros(T, dtype=np.float32)
        # per-slot degree (0 for dummies)
        slot_deg = np.zeros(n_slot, dtype=np.int64)
        slot_deg[obj_slot] = deg
        # destination token index for each sorted edge:
        # chunk-level base for the slot + within-object rank
        slot_tok0 = np.zeros(n_slot, dtype=np.int64)
        for (tok0, ntok, nobj, R, sl0) in chunks:
            slot_tok0[sl0:sl0 + nobj] = tok0 + np.arange(nobj) * R
        within = np.zeros(len(order), dtype=np.int64)
        if len(order):
            boundaries = np.nonzero(np.diff(s_slot))[0] + 1
            starts = np.zeros(len(order), dtype=np.int64)
            starts[boundaries] = boundaries
            starts = np.maximum.accumulate(starts)
            within = np.arange(len(order)) - starts
        dst = slot_tok0[s_slot] + within
        tok_attr[dst] = s_lattr.astype(np.int16)
        tok_w[dst] = s_wp

        # wrapped [16, T/16], replicated to all 8 GPSIMD core groups -> [128, T/16]
        widx16 = tok_attr.reshape(T // 16, 16).T
        widx = np.ascontiguousarray(np.tile(widx16, (8, 1)))
        wtok = tok_w.astype(_BF16)                     # [T] bf16

        xperm = np.zeros((n_slot, IN_DIM), dtype=np.float32)
        real = perm >= 0
        xperm[real] = X[c][perm[real]]

        im = {
            "attr_re": attr_re,
            "xperm": xperm,
            "widx": widx,
            "wtok": wtok,
            "w1t0": w1t_bf[0],
            "w1t1": w1t_bf[1],
            "wct": wct_bf,
        }
        if has_bias:
            ind = np.zeros((2, n_slot), dtype=_BF16)
            ind[0, :] = ((ws > 1e-6)[perm.clip(0)] & real).astype(_BF16)
            ind[1, :] = real.astype(_BF16)
            im["bias_lhsT"] = ind
            im["bias_rhs"] = bias_rhs
        in_maps.append(im)
        metas.append(perm)

    structure = (tuple(r_list), tuple(n_per_bucket), has_bias)
    return structure, (chunks, T, n_slot, has_bias), in_maps, metas


# ================================================================ bass build
def _build(chunks, T, n_slot, has_bias, skip=()):
    nc = bacc.Bacc(None, target_bir_lowering=False, debug=False)
    f32 = mybir.dt.float32
    bf16 = mybir.dt.bfloat16

    attr_re = nc.declare_dram_parameter(
        "attr_re", [128, (NA_TOT // 128) * ATTR_DIM], bf16, isOutput=False)
    xperm = nc.declare_dram_parameter("xperm", [n_slot, IN_DIM], f32, isOutput=False)
    widx = nc.declare_dram_parameter("widx", [128, T // 16], mybir.dt.int16, isOutput=False)
    wtok = nc.declare_dram_parameter("wtok", [T], bf16, isOutput=False)
    w1t0 = nc.declare_dram_parameter("w1t0", [128, OUT_DIM], bf16, isOutput=False)
    w1t1 = nc.declare_dram_parameter("w1t1", [128, OUT_DIM], bf16, isOutput=False)
    wct = nc.declare_dram_parameter("wct", [ATTR_DIM, OUT_DIM], bf16, isOutput=False)
    if has_bias:
        bias_lhsT = nc.declare_dram_parameter("bias_lhsT", [2, n_slot], bf16, isOutput=False)
        bias_rhs = nc.declare_dram_parameter("bias_rhs", [2, OUT_DIM], bf16, isOutput=False)
    out = nc.declare_dram_parameter("out", [n_slot, OUT_DIM], f32, isOutput=True)

    n_ranks = NA_TOT // 128  # 125

    with tile.TileContext(nc) as tc:
        with (
            tc.tile_pool(name="const", bufs=1) as constp,
            tc.tile_pool(name="big", bufs=1) as bigp,
            tc.tile_pool(name="work", bufs=3) as workp,
        ):
            # ---------------- persistent SBUF tensors
            table = constp.tile([128, NA_TOT], bf16, tag="table")
            xT = bigp.tile([128, 2, n_slot], bf16, tag="xT")
            sT = bigp.tile([128, n_slot], f32, tag="sT")
            sTbf = bigp.tile([128, n_slot], bf16, tag="sTbf")
            ident = constp.tile([128, 128], bf16, tag="ident")
            ones_row = constp.tile([1, 128], bf16, tag="ones")
            w1t0_sb = constp.tile([128, OUT_DIM], bf16, tag="w1t0")
            w1t1_sb = constp.tile([128, OUT_DIM], bf16, tag="w1t1")
            wct_sb = constp.tile([128, OUT_DIM], bf16, tag="wct")
            make_identity(nc, ident[:])
            nc.gpsimd.memset(ones_row[:], 1.0)
            nc.sync.dma_start(w1t0_sb[:], w1t0[:])
            nc.sync.dma_start(w1t1_sb[:], w1t1[:])
            nc.sync.dma_start(wct_sb[:], wct[:])
            if has_bias:
                bias_lhsT_sb = constp.tile([2, n_slot], bf16, tag="biasl")
                bias_rhs_sb = constp.tile([2, OUT_DIM], bf16, tag="biasr")
                nc.sync.dma_start(bias_lhsT_sb[:], bias_lhsT[:])
                nc.sync.dma_start(bias_rhs_sb[:], bias_rhs[:])

            # ---------------- phase A: attr table load (host already bf16)
            nc.sync.dma_start(table[:], attr_re[:])

            # ---------------- phase B: X -> bf16 -> transpose -> xT
            with tc.tile_pool(name="xpsum", bufs=2, space="PSUM") as xpp:
                for k0 in ([] if "phaseB" in skip else range(0, n_slot, 128)):
                    xf = workp.tile([128, IN_DIM], f32, tag="xf")
                    xb = workp.tile([128, IN_DIM], bf16, tag="xb")
                    nc.sync.dma_start(xf[:], xperm[k0:k0 + 128, :])
                    nc.scalar.copy(xb[:], xf[:])
                    for j in range(2):
                        pt = xpp.tile([128, 128], bf16, tag="xtp")
                        nc.tensor.transpose(pt[:], xb[:, j * 128:(j + 1) * 128], ident[:])
                        nc.scalar.copy(xT[:, j, k0:k0 + 128], pt[:])

            # ---------------- phase C: gather + weight + segmented reduce
            with tc.tile_pool(name="wps", bufs=2, space="PSUM") as wpsp:
                for (tok0, ntok, nobj, R, slot0) in ([] if "phaseC" in skip else chunks):
                    idxt = workp.tile([128, CHUNK_TOK // 16], mybir.dt.int16, tag="idx")
                    nc.sync.dma_start(idxt[:, :ntok // 16],
                                      widx[:, tok0 // 16:(tok0 + ntok) // 16])
                    g = workp.tile([128, CHUNK_TOK], bf16, tag="g")
                    if "gather" not in skip:
                        nc.gpsimd.dma_gather(
                            g[:, :ntok].rearrange("p (one t) -> p one t", one=1),
                            table[:],
                            idxt[:, :ntok // 16],
                            num_idxs=ntok,
                            num_idxs_reg=ntok,
                            elem_size=ATTR_DIM,
                            transpose=True,
                            sbuf_tokens_per_rank=128,
                            sbuf_free_dim_per_rank=ATTR_DIM * 2,
                            single_packet=False,
                        )
                    wrow = workp.tile([1, CHUNK_TOK], bf16, tag="wrow")
                    nc.sync.dma_start(wrow[0:1, :ntok], wtok[None, tok0:tok0 + ntok])
                    wps = wpsp.tile([128, CHUNK_TOK], f32, tag="wps")
                    if "wrep" not in skip:
                        for j0 in range(0, ntok, 512):
                            n = min(512, ntok - j0)
                            nc.tensor.matmul(
                                wps[:, j0:j0 + n], ones_row[:],
                                wrow[0:1, j0:j0 + n], start=True, stop=True)
                    wx = workp.tile([128, CHUNK_TOK], bf16, tag="wx")
                    if "mult" not in skip:
                        nc.vector.tensor_tensor(
                            wx[:, :ntok], g[:, :ntok], wps[:, :ntok],
                            mybir.AluOpType.mult)
                    if "reduce" not in skip:
                        nc.vector.tensor_reduce(
                            sT[:, slot0:slot0 + nobj],
                            wx[:, :nobj * R].rearrange("p (o r) -> p o r", r=R),
                            axis=mybir.AxisListType.X,
                            op=mybir.AluOpType.add)

            # bulk convert sT -> bf16 for the dense matmul
            if not ({"phaseC", "reduce", "phaseD"} & set(skip)):
                nc.scalar.copy(sTbf[:], sT[:])

            # ---------------- phase D: dense matmuls + relu + store
            with tc.tile_pool(name="eppsum", bufs=2, space="PSUM") as eppp:
                for k0 in ([] if ("phaseD" in skip or "phaseB" in skip or
                                  "phaseC" in skip or "reduce" in skip)
                           else range(0, n_slot, 128)):
                    po = eppp.tile([128, OUT_DIM], f32, tag="po")
                    nc.tensor.matmul(po[:], xT[:, 0, k0:k0 + 128], w1t0_sb[:],
                                     start=True, stop=False)
                    nc.tensor.matmul(po[:], xT[:, 1, k0:k0 + 128], w1t1_sb[:],
                                     start=False, stop=False)
                    nc.tensor.matmul(po[:], sTbf[:, k0:k0 + 128], wct_sb[:],
                                     start=False, stop=not has_bias)
                    if has_bias:
                        nc.tensor.matmul(po[:], bias_lhsT_sb[:, k0:k0 + 128],
                                         bias_rhs_sb[:], start=False, stop=True)
                    ob = workp.tile([128, OUT_DIM], f32, tag="ob")
                    nc.scalar.activation(ob[:], po[:],
                                         mybir.ActivationFunctionType.Relu)
                    nc.sync.dma_start(out[k0:k0 + 128, :], ob[:])

    nc.finalize()
    return nc


# ================================================================ v2: scan-based
OBJ_CHUNK = 512          # objects per chunk (20 chunks cover 10240 slots)
N_SLOT2 = 10240
SUB_TOK = 2048           # sub-chunk size for W_rep psum / mult / scan
DMA_GATHER_FRAC = 1, 1   # route chunk k to dma_gather if (k*a)%b < a


def _weights_prep(W_a2o, b_a2o, W_proj, b_proj, W_upd, b_upd):
    W_upd64 = np.asarray(W_upd, dtype=np.float64)
    W1 = W_upd64[:, :IN_DIM]
    W2 = W_upd64[:, IN_DIM:]
    Wp = np.asarray(W_proj, dtype=np.float64)
    Wa = np.asarray(W_a2o, dtype=np.float64)
    Wc = W2 @ Wp @ Wa
    ba2 = W2 @ Wp @ np.asarray(b_a2o, dtype=np.float64)
    bconst = W2 @ np.asarray(b_proj, dtype=np.float64) + np.asarray(b_upd, np.float64)
    has_bias = bool(np.any(ba2 != 0.0) or np.any(bconst != 0.0))
    w1t_bf = np.ascontiguousarray(W1.T.reshape(2, 128, OUT_DIM).astype(_BF16))
    wct_bf = np.ascontiguousarray(Wc.T.astype(_BF16))
    bias_rhs = np.stack([ba2, bconst]).astype(_BF16)
    return w1t_bf, wct_bf, bias_rhs, has_bias


def _prepare2(object_feats, attr_feats, edge_index, edge_weight,
              W_a2o, b_a2o, W_proj, b_proj, W_upd, b_upd):
    src_obj = np.asarray(edge_index[0]).astype(np.int64)
    src_attr = np.asarray(edge_index[1]).astype(np.int64)
    w = np.asarray(edge_weight, dtype=np.float32)
    X = np.asarray(object_feats, dtype=np.float32)
    A = np.asarray(attr_feats, dtype=np.float32).reshape(NA_TOT, ATTR_DIM)

    w1t_bf, wct_bf, bias_rhs, has_bias = _weights_prep(
        W_a2o, b_a2o, W_proj, b_proj, W_upd, b_upd)

    core_of = src_obj // N_OBJ
    per_core = []
    for c in range(NCORES):
        m = core_of == c
        lobj = (src_obj[m] - c * N_OBJ).astype(np.int64)
        per_core.append((lobj, src_attr[m].astype(np.int64), w[m]))

    nchunks = N_SLOT2 // OBJ_CHUNK
    # shared static token capacity per chunk
    cap = 0
    degs = []
    for c in range(NCORES):
        deg = np.bincount(per_core[c][0], minlength=N_SLOT2)
        degs.append(deg)
        per_chunk = deg.reshape(nchunks, OBJ_CHUNK).sum(axis=1)
        cap = max(cap, int(per_chunk.max()))
    cap = -(-(cap) // 128) * 128
    T = nchunks * cap

    # row-wrap layout for dma_gather (row i at partition i%128, rank i//128)
    attr_re = np.ascontiguousarray(
        A.reshape(NA_TOT // 128, 128, ATTR_DIM).transpose(1, 0, 2)
        .reshape(128, (NA_TOT // 128) * ATTR_DIM).astype(_BF16))
    # transposed layout for indirect_copy (dim on partitions)
    attr_t = np.ascontiguousarray(A.T.astype(_BF16))

    in_maps = []
    for c in range(NCORES):
        lobj, lattr, wv = per_core[c]
        deg = degs[c]
        ws = np.zeros(N_SLOT2, dtype=np.float64)
        np.add.at(ws, lobj, wv.astype(np.float64))
        assert not np.any((ws > 0) & (ws <= 2e-6)), "r1 edge case hit"
        wprime = (wv / np.maximum(ws, 1e-6)[lobj]).astype(np.float32)

        order = np.argsort(lobj, kind="stable")
        s_attr = lattr[order]
        s_w = wprime[order]
        cum = np.concatenate([[0], np.cumsum(deg)])

        tok_attr = np.zeros(T, dtype=np.int16)
        tok_w = np.zeros(T, dtype=np.float32)
        bends = np.zeros((nchunks, OBJ_CHUNK), dtype=np.int16)
        for k in range(nchunks):
            o0 = k * OBJ_CHUNK
            e0, e1 = cum[o0], cum[o0 + OBJ_CHUNK]
            n = e1 - e0
            assert n <= cap
            tok_attr[k * cap:k * cap + n] = s_attr[e0:e1]
            tok_w[k * cap:k * cap + n] = s_w[e0:e1]
            bends[k] = (cum[o0 + 1:o0 + OBJ_CHUNK + 1] - e0).astype(np.int16)

        widx = np.ascontiguousarray(
            np.tile(tok_attr.reshape(T // 16, 16).T, (8, 1)))
        bidx = np.ascontiguousarray(
            np.tile(bends.reshape(nchunks * OBJ_CHUNK // 16, 16).T, (8, 1)))
        wtok = tok_w.astype(_BF16)

        xt = np.zeros((128, 2, N_SLOT2), dtype=_BF16)
        xt[:, 0, :N_OBJ] = X[c][:, :128].T
        xt[:, 1, :N_OBJ] = X[c][:, 128:].T

        im = {
            "attr_re": attr_re,
            "attr_t": attr_t,
            "xt": np.ascontiguousarray(xt.reshape(128, 2 * N_SLOT2)),
            "widx": widx,
            "bidx": bidx,
            "wtok": wtok,
            "w1t0": w1t_bf[0],
            "w1t1": w1t_bf[1],
            "wct": wct_bf,
        }
        if has_bias:
            ind = np.zeros((2, N_SLOT2), dtype=_BF16)
            ind[0, :] = (ws > 1e-6).astype(_BF16)
            ind[1, :N_OBJ] = 1.0
            im["bias_lhsT"] = ind
            im["bias_rhs"] = bias_rhs
        in_maps.append(im)

    structure = ("v2", cap, has_bias)
    return structure, (cap, T, has_bias), in_maps


def _build2(cap, T, has_bias, dma_frac=DMA_GATHER_FRAC, repeat=1,
            wrep_evict=False, relu_dve=False, nqueues=1, skip=()):
    nc = bacc.Bacc(None, target_bir_lowering=False, debug=False,
                   num_swdge_queues=nqueues)
    f32 = mybir.dt.float32
    bf16 = mybir.dt.bfloat16
    i16 = mybir.dt.int16
    nchunks = N_SLOT2 // OBJ_CHUNK

    attr_re = nc.declare_dram_parameter(
        "attr_re", [128, (NA_TOT // 128) * ATTR_DIM], bf16, isOutput=False)
    attr_t = nc.declare_dram_parameter("attr_t", [128, NA_TOT], bf16, isOutput=False)
    xt_in = nc.declare_dram_parameter("xt", [128, 2 * N_SLOT2], bf16, isOutput=False)
    widx = nc.declare_dram_parameter("widx", [128, T // 16], i16, isOutput=False)
    bidx = nc.declare_dram_parameter("bidx", [128, N_SLOT2 // 16], i16, isOutput=False)
    wtok = nc.declare_dram_parameter("wtok", [T], bf16, isOutput=False)
    w1t0 = nc.declare_dram_parameter("w1t0", [128, OUT_DIM], bf16, isOutput=False)
    w1t1 = nc.declare_dram_parameter("w1t1", [128, OUT_DIM], bf16, isOutput=False)
    wct = nc.declare_dram_parameter("wct", [ATTR_DIM, OUT_DIM], bf16, isOutput=False)
    if has_bias:
        bias_lhsT = nc.declare_dram_parameter("bias_lhsT", [2, N_SLOT2], bf16, isOutput=False)
        bias_rhs = nc.declare_dram_parameter("bias_rhs", [2, OUT_DIM], bf16, isOutput=False)
    out = nc.declare_dram_parameter("out", [N_SLOT2, OUT_DIM], f32, isOutput=True)

    with tile.TileContext(nc) as tc:
        with (
            tc.tile_pool(name="const", bufs=1) as constp,
            tc.tile_pool(name="big", bufs=1) as bigp,
            tc.tile_pool(name="work", bufs=3) as workp,
            tc.tile_pool(name="gp", bufs=2) as gpool,
            tc.tile_pool(name="pfx", bufs=1) as pfxp,
        ):
            use_pool_gather = dma_frac[0] < dma_frac[1]
            table_r = constp.tile([128, NA_TOT], bf16, tag="table_r")
            table_t = (constp.tile([128, NA_TOT], bf16, tag="table_t")
                       if use_pool_gather else None)
            sTbf = bigp.tile([128, N_SLOT2], bf16, tag="sTbf")
            ones_row = constp.tile([1, 128], bf16, tag="ones")
            w1t0_sb = constp.tile([128, OUT_DIM], bf16, tag="w1t0")
            w1t1_sb = constp.tile([128, OUT_DIM], bf16, tag="w1t1")
            wct_sb = constp.tile([128, OUT_DIM], bf16, tag="wct")
            bidx_sb = constp.tile([128, N_SLOT2 // 16], i16, tag="bidx")
            nc.gpsimd.memset(ones_row[:], 1.0)
            nc.sync.dma_start(table_r[:], attr_re[:])
            if use_pool_gather:
                nc.sync.dma_start(table_t[:], attr_t[:])
            nc.sync.dma_start(w1t0_sb[:], w1t0[:])
            nc.sync.dma_start(w1t1_sb[:], w1t1[:])
            nc.sync.dma_start(wct_sb[:], wct[:])
            nc.sync.dma_start(bidx_sb[:], bidx[:])
            if has_bias:
                bias_lhsT_sb = constp.tile([2, N_SLOT2], bf16, tag="biasl")
                bias_rhs_sb = constp.tile([2, OUT_DIM], bf16, tag="biasr")
                nc.sync.dma_start(bias_lhsT_sb[:], bias_lhsT[:])
                nc.sync.dma_start(bias_rhs_sb[:], bias_rhs[:])

            zero_col = constp.tile([128, 1], f32, tag="zcol")
            nc.vector.memset(zero_col[:], 0.0)
            if ("phaseC" in skip or "subtract" in skip) and "phaseD" not in skip:
                nc.vector.memset(sTbf[:], 0.0)

            # ---------------- phase C: gather + weight + scan + boundary diff
            with tc.tile_pool(name="wps", bufs=2, space="PSUM") as wpsp:
                for k in [kk for _ in range(0 if "phaseC" in skip else repeat)
                          for kk in range(nchunks)]:
                    tok0 = k * cap
                    use_dma = (k * dma_frac[0]) % dma_frac[1] < dma_frac[0]
                    g = gpool.tile([128, cap], bf16, tag="g")
                    if "gather" in skip:
                        pass
                    elif use_dma:
                        idxt = workp.tile([128, cap // 16], i16, tag="idx")
                        nc.sync.dma_start(idxt[:],
                                          widx[:, tok0 // 16:(tok0 + cap) // 16])
                        nc.gpsimd.dma_gather(
                            g[:].rearrange("p (one t) -> p one t", one=1),
                            table_r[:],
                            idxt[:],
                            num_idxs=cap,
                            num_idxs_reg=cap,
                            elem_size=ATTR_DIM,
                            transpose=True,
                            sbuf_tokens_per_rank=128,
                            sbuf_free_dim_per_rank=ATTR_DIM * 2,
                            single_packet=False,
                            queue_num=k % nqueues,
                        )
                    else:
                        idxt = workp.tile([128, cap // 16], i16, tag="idx")
                        nc.sync.dma_start(idxt[:],
                                          widx[:, tok0 // 16:(tok0 + cap) // 16])
                        for j0 in range(0, cap, SUB_TOK):
                            n = min(SUB_TOK, cap - j0)
                            nc.gpsimd.indirect_copy(
                                g[:, j0:j0 + n], table_t[:],
                                idxt[:, j0 // 16:(j0 + n) // 16].bitcast(mybir.dt.uint16),
                                i_know_ap_gather_is_preferred=True)

                    # exclusive prefix: prefix[:, j] = sum of tokens < j
                    prefix = pfxp.tile([128, 1 + cap], f32, tag="prefix")
                    nc.vector.memset(prefix[:, 0:1], 0.0)
                    for j0 in range(0, cap, SUB_TOK):
                        n = min(SUB_TOK, cap - j0)
                        wrow = workp.tile([1, SUB_TOK], bf16, tag="wrow")
                        nc.sync.dma_start(wrow[0:1, :n],
                                          wtok[None, tok0 + j0:tok0 + j0 + n])
                        wps = wpsp.tile([128, SUB_TOK], f32, tag="wps")
                        if "wrep" not in skip:
                            for i0 in range(0, n, 512):
                                m = min(512, n - i0)
                                nc.tensor.matmul(
                                    wps[:, i0:i0 + m], ones_row[:],
                                    wrow[0:1, i0:i0 + m], start=True, stop=True)
                        wx = workp.tile([128, SUB_TOK], bf16, tag="wx")
                        if "mult" in skip:
                            pass
                        elif wrep_evict:
                            wrb = workp.tile([128, SUB_TOK], bf16, tag="wrb")
                            nc.scalar.copy(wrb[:, :n], wps[:, :n])
                            nc.vector.scalar_tensor_tensor(
                                out=wx[:, :n], in0=g[:, j0:j0 + n], scalar=1.0,
                                in1=wrb[:, :n], op0=mybir.AluOpType.mult,
                                op1=mybir.AluOpType.mult)
                        else:
                            nc.vector.scalar_tensor_tensor(
                                out=wx[:, :n], in0=g[:, j0:j0 + n], scalar=1.0,
                                in1=wps[:, :n], op0=mybir.AluOpType.mult,
                                op1=mybir.AluOpType.mult)
                        if "scan" not in skip:
                            init = 0.0 if j0 == 0 else prefix[:, j0:j0 + 1]
                            nc.vector.tensor_tensor_scan(
                                prefix[:, 1 + j0:1 + j0 + n], wx[:, :n],
                                zero_col[:].to_broadcast([128, n]),
                                initial=init,
                                op0=mybir.AluOpType.add, op1=mybir.AluOpType.add)

                    gbuf = workp.tile([128, OBJ_CHUNK + 16], f32, tag="gbuf")
                    nc.vector.memset(gbuf[:, 0:1], 0.0)
                    if "apgather" not in skip:
                        nc.gpsimd.ap_gather(
                            gbuf[:, 1:1 + OBJ_CHUNK].rearrange("p (n one) -> p n one", one=1),
                            prefix[:].rearrange("p (n one) -> p n one", one=1),
                            bidx_sb[:, k * OBJ_CHUNK // 16:(k + 1) * OBJ_CHUNK // 16],
                            channels=128, num_elems=1 + cap, d=1, num_idxs=OBJ_CHUNK)
                    if "subtract" not in skip:
                        nc.vector.tensor_tensor(
                            sTbf[:, k * OBJ_CHUNK:(k + 1) * OBJ_CHUNK],
                            gbuf[:, 1:1 + OBJ_CHUNK], gbuf[:, 0:OBJ_CHUNK],
                            mybir.AluOpType.subtract)

            # ---------------- phase D: dense matmuls + relu + store
            with tc.tile_pool(name="eppsum", bufs=4, space="PSUM") as eppp:
                for k0 in [kk for _ in range(0 if "phaseD" in skip else repeat)
                           for kk in range(0, N_SLOT2, 128)]:
                    xa = workp.tile([128, 2, 128], bf16, tag="xa")
                    nc.sync.dma_start(
                        xa[:],
                        xt_in[:].rearrange("p (a b) -> p a b", a=2)[:, :, k0:k0 + 128])
                    po = eppp.tile([128, OUT_DIM], f32, tag="po")
                    nc.tensor.matmul(po[:], xa[:, 0, :], w1t0_sb[:],
                                     start=True, stop=False)
                    nc.tensor.matmul(po[:], xa[:, 1, :], w1t1_sb[:],
                                     start=False, stop=False)
                    nc.tensor.matmul(po[:], sTbf[:, k0:k0 + 128], wct_sb[:],
                                     start=False, stop=not has_bias)
                    if has_bias:
                        nc.tensor.matmul(po[:], bias_lhsT_sb[:, k0:k0 + 128],
                                         bias_rhs_sb[:], start=False, stop=True)
                    ob = workp.tile([128, OUT_DIM], f32, tag="ob")
                    if relu_dve:
                        nc.vector.tensor_scalar_max(ob[:], po[:], 0.0)
                    else:
                        nc.scalar.activation(ob[:], po[:],
                                             mybir.ActivationFunctionType.Relu)
                    nc.sync.dma_start(out[k0:k0 + 128, :], ob[:])

    nc.finalize()
    return nc


def kernel2(object_feats, attr_feats, edge_index, edge_weight,
            W_a2o, b_a2o, W_proj, b_proj, W_upd, b_upd):
    structure, plan, in_maps = _prepare2(
        object_feats, attr_feats, edge_index, edge_weight,
        W_a2o, b_a2o, W_proj, b_proj, W_upd, b_upd)
    cap, T, has_bias = plan
    nc = _compiled_cache.get(structure)
    if nc is None:
        nc = _build2(cap, T, has_bias)
        _compiled_cache[structure] = nc
    res = run_bass_kernel_spmd(nc, in_maps, list(range(NCORES))).results
    out = np.stack([res[c]["out"][:N_OBJ] for c in range(NCORES)])
    return np.ascontiguousarray(out).reshape(B, N_OBJ, OUT_DIM)


# ================================================================ v3: PE-matmul segsum
# Layout: per-core objects degree-sorted into slots; slot capacities are the
# elementwise max of the 8 cores' sorted degree profiles (static, data-derived,
# shared by all cores).  Tokens (edges, padded to capacity) stream in slot
# order; each 128-token block is gathered in natural layout ([token, attr]) by
# a DRAM-source dma_gather and reduced into per-slot sums by a single PE
# matmul against a host-built selection matrix sel[token, slot_window] that
# carries the normalized edge weights.  No DVE/Pool work in the inner loop.
N_SLOT3 = 10112          # 10000 real slots + pad to a multiple of 128
TOKCAP3 = 6144           # max tokens per chunk (psum window: <=512 slots)


def _prepare3(object_feats, attr_feats, edge_index, edge_weight,
              W_a2o, b_a2o, W_proj, b_proj, W_upd, b_upd):
    src_obj = np.asarray(edge_index[0]).astype(np.int64)
    src_attr = np.asarray(edge_index[1]).astype(np.int64)
    w = np.asarray(edge_weight, dtype=np.float32)
    X = np.asarray(object_feats, dtype=np.float32)
    A = np.asarray(attr_feats, dtype=np.float32).reshape(NA_TOT, ATTR_DIM)

    w1t_bf, wct_bf, bias_rhs, has_bias = _weights_prep(
        W_a2o, b_a2o, W_proj, b_proj, W_upd, b_upd)

    core_of = src_obj // N_OBJ
    per_core = []
    degs = []
    perms = []
    for c in range(NCORES):
        m = core_of == c
        lobj = (src_obj[m] - c * N_OBJ).astype(np.int64)
        per_core.append((lobj, src_attr[m].astype(np.int64), w[m]))
        deg = np.bincount(lobj, minlength=N_OBJ)
        degs.append(deg)
        perms.append(np.argsort(-deg, kind="stable"))

    # ---- static slot-capacity profile (shared across cores) ----
    sorted_deg = np.stack([degs[c][perms[c]] for c in range(NCORES)])
    cap = sorted_deg.max(axis=0).astype(np.int64)
    capp = np.concatenate([cap, np.zeros(N_SLOT3 - N_OBJ, np.int64)])

    # ---- chunks: consecutive slots, <=512 slots and <=TOKCAP3 tokens ----
    chunks = []  # (s0, s1, tok0, ntok)
    s0 = 0
    tok0 = 0
    while s0 < N_SLOT3:
        s1 = s0
        t = 0
        while s1 < N_SLOT3 and s1 - s0 < 512 and t + capp[s1] <= TOKCAP3:
            t += int(capp[s1])
            s1 += 1
        ntok = -(-t // 128) * 128
        chunks.append((s0, s1, tok0, ntok))
        tok0 += ntok
        s0 = s1
    T = tok0

    # ---- static slot->token map and block windows ----
    slot_tok0 = np.zeros(N_SLOT3 + 1, dtype=np.int64)
    slot_of_tok = np.zeros(T, dtype=np.int64)
    for (s0, s1, tok0, ntok) in chunks:
        cs = np.concatenate([[0], np.cumsum(capp[s0:s1])])
        slot_tok0[s0:s1] = tok0 + cs[:-1]
        sl = np.repeat(np.arange(s0, s1), capp[s0:s1])
        sl = np.concatenate(
            [sl, np.full(ntok - len(sl), max(s1 - 1, s0), np.int64)])
        slot_of_tok[tok0:tok0 + ntok] = sl
    nblk = T // 128
    blk_first = slot_of_tok.reshape(nblk, 128)[:, 0]
    blk_last = slot_of_tok.reshape(nblk, 128)[:, -1]
    blk_W = (blk_last - blk_first + 1).astype(np.int64)
    colptr = np.concatenate([[0], np.cumsum(blk_W)]).astype(np.int64)
    ncols = int(colptr[-1])

    # per-chunk block lists: (local_block, psum_col0, W, chunk_col0)
    blk_of_chunk = []
    chunk_cols = []
    for (s0, s1, tok0, ntok) in chunks:
        b0 = tok0 // 128
        b1 = (tok0 + ntok) // 128
        cks = []
        for b in range(b0, b1):
            cks.append((b - b0, int(blk_first[b] - s0), int(blk_W[b]),
                        int(colptr[b] - colptr[b0])))
        blk_of_chunk.append(tuple(cks))
        chunk_cols.append((int(colptr[b0]), int(colptr[b1] - colptr[b0])))
    max_ck = max((c[1] for c in chunk_cols), default=1)

    attr_nat = np.ascontiguousarray(A.astype(_BF16))

    in_maps = []
    for c in range(NCORES):
        lobj, lattr, wv = per_core[c]
        deg = degs[c]
        perm = perms[c]
        ws = np.zeros(N_OBJ, dtype=np.float64)
        np.add.at(ws, lobj, wv.astype(np.float64))
        assert not np.any((ws > 0) & (ws <= 2e-6)), "r1 edge case hit"
        wprime = (wv / np.maximum(ws, 1e-6)[lobj]).astype(np.float32)

        slot_of_obj = np.empty(N_OBJ, dtype=np.int64)
        slot_of_obj[perm] = np.arange(N_OBJ)
        order = np.argsort(slot_of_obj[lobj], kind="stable")
        s_attr = lattr[order]
        s_w = wprime[order]
        s_slot = slot_of_obj[lobj][order]

        # within-slot rank of each sorted edge
        n = len(order)
        within = np.zeros(n, dtype=np.int64)
        if n:
            boundaries = np.nonzero(np.diff(s_slot))[0] + 1
            starts = np.zeros(n, dtype=np.int64)
            starts[boundaries] = boundaries
            starts = np.maximum.accumulate(starts)
            within = np.arange(n) - starts
        dst = slot_tok0[s_slot] + within

        tok_attr = np.zeros(T, dtype=np.int16)
        tok_attr[dst] = s_attr.astype(np.int16)
        widx = np.ascontiguousarray(
            np.tile(tok_attr.reshape(T // 16, 16).T, (8, 1)))

        sel = np.zeros((128, ncols), dtype=_BF16)
        blk = dst // 128
        part = dst % 128
        col = colptr[blk] + (s_slot - blk_first[blk])
        sel[part, col] = s_w

        xt = np.zeros((128, 2, N_SLOT3), dtype=_BF16)
        xt[:, 0, :N_OBJ] = X[c][perm, :128].T
        xt[:, 1, :N_OBJ] = X[c][perm, 128:].T

        im = {
            "attr_nat": attr_nat,
            "xt": np.ascontiguousarray(xt.reshape(128, 2 * N_SLOT3)),
            "widx": widx,
            "sel": sel,
            "w1t0": w1t_bf[0],
            "w1t1": w1t_bf[1],
            "wct": wct_bf,
        }
        if has_bias:
            ind = np.zeros((2, N_SLOT3), dtype=_BF16)
            ind[0, :N_OBJ] = (ws > 1e-6)[perm].astype(_BF16)
            ind[1, :N_OBJ] = 1.0
            im["bias_lhsT"] = ind
            im["bias_rhs"] = bias_rhs
        in_maps.append(im)

    structure = ("v3", T, ncols, has_bias, tuple(chunks),
                 tuple(blk_W.tolist()), tuple(blk_first.tolist()))
    plan = (tuple(chunks), blk_of_chunk, tuple(chunk_cols), max_ck, T, ncols,
            has_bias)
    return structure, plan, in_maps, perms


def _build3(chunks, blk_of_chunk, chunk_cols, max_ck, T, ncols, has_bias,
            nqueues=4, skip=(), repeat=1, piece=1024, single_packet=True):
    nc = bacc.Bacc(None, target_bir_lowering=False, debug=False,
                   num_swdge_queues=nqueues)
    f32 = mybir.dt.float32
    bf16 = mybir.dt.bfloat16
    i16 = mybir.dt.int16

    attr_nat = nc.declare_dram_parameter(
        "attr_nat", [NA_TOT, ATTR_DIM], bf16, isOutput=False)
    xt_in = nc.declare_dram_parameter("xt", [128, 2 * N_SLOT3], bf16, isOutput=False)
    widx = nc.declare_dram_parameter("widx", [128, T // 16], i16, isOutput=False)
    sel_in = nc.declare_dram_parameter("sel", [128, ncols], bf16, isOutput=False)
    w1t0 = nc.declare_dram_parameter("w1t0", [128, OUT_DIM], bf16, isOutput=False)
    w1t1 = nc.declare_dram_parameter("w1t1", [128, OUT_DIM], bf16, isOutput=False)
    wct = nc.declare_dram_parameter("wct", [ATTR_DIM, OUT_DIM], bf16, isOutput=False)
    if has_bias:
        bias_lhsT = nc.declare_dram_parameter("bias_lhsT", [2, N_SLOT3], bf16, isOutput=False)
        bias_rhs = nc.declare_dram_parameter("bias_rhs", [2, OUT_DIM], bf16, isOutput=False)
    out = nc.declare_dram_parameter("out", [N_SLOT3, OUT_DIM], f32, isOutput=True)

    with tile.TileContext(nc) as tc:
        with (
            tc.tile_pool(name="const", bufs=1) as constp,
            tc.tile_pool(name="big", bufs=1) as bigp,
            tc.tile_pool(name="work", bufs=3) as workp,
            tc.tile_pool(name="gp", bufs=2) as gpool,
            tc.tile_pool(name="selp", bufs=2) as selp,
        ):
            sTbf = bigp.tile([128, N_SLOT3], bf16, tag="sTbf")
            zrow = constp.tile([1, 128], bf16, tag="zrow")
            zcols = constp.tile([1, 512], bf16, tag="zcols")
            w1t0_sb = constp.tile([128, OUT_DIM], bf16, tag="w1t0")
            w1t1_sb = constp.tile([128, OUT_DIM], bf16, tag="w1t1")
            wct_sb = constp.tile([128, OUT_DIM], bf16, tag="wct")
            nc.gpsimd.memset(zrow[:], 0.0)
            nc.gpsimd.memset(zcols[:], 0.0)
            nc.sync.dma_start(w1t0_sb[:], w1t0[:])
            nc.sync.dma_start(w1t1_sb[:], w1t1[:])
            nc.sync.dma_start(wct_sb[:], wct[:])
            if has_bias:
                bias_lhsT_sb = constp.tile([2, N_SLOT3], bf16, tag="biasl")
                bias_rhs_sb = constp.tile([2, OUT_DIM], bf16, tag="biasr")
                nc.sync.dma_start(bias_lhsT_sb[:], bias_lhsT[:])
                nc.sync.dma_start(bias_rhs_sb[:], bias_rhs[:])

            # ---------------- phase C: gather + PE segsum ----------------
            gi = 0  # gather emission counter; keeps SWDGE sem lane <-> queue fixed
            with tc.tile_pool(name="cps", bufs=2, space="PSUM") as cpp:
                for k, (s0, s1, tok0, ntok) in (
                        [] if "phaseC" in skip
                        else [kv for _ in range(repeat)
                              for kv in enumerate(chunks)]):
                    nsl = s1 - s0
                    ps = cpp.tile([128, 512], f32, tag="ps")
                    nc.tensor.matmul(ps[:, :nsl], zrow[:], zcols[0:1, :nsl],
                                     start=True, stop=False)
                    if ntok and "gather" not in skip:
                        idxt = workp.tile([128, TOKCAP3 // 16], i16, tag="idx")
                        nc.sync.dma_start(
                            idxt[:, :ntok // 16],
                            widx[:, tok0 // 16:(tok0 + ntok) // 16])
                        g = gpool.tile([128, TOKCAP3], bf16, tag="g")
                        gv = g[:].rearrange("p (b e) -> p b e", e=ATTR_DIM)
                        # SWDGE ring holds ~1024 descriptors; split the gather
                        for t0 in range(0, ntok, piece):
                            n = min(piece, ntok - t0)
                            nc.gpsimd.dma_gather(
                                gv[:, t0 // 128:(t0 + n) // 128, :],
                                attr_nat[:],
                                idxt[:, t0 // 16:(t0 + n) // 16],
                                num_idxs=n,
                                num_idxs_reg=n,
                                elem_size=ATTR_DIM,
                                transpose=False,
                                single_packet=single_packet,
                                queue_num=gi % nqueues,
                            )
                            gi += 1
                        c0, ck = chunk_cols[k]
                        csel = selp.tile([128, max_ck], bf16, tag="csel")
                        nc.sync.dma_start(csel[:, :ck], sel_in[:, c0:c0 + ck])
                        if "segmm" not in skip:
                            for (lb, w0, W, cc) in blk_of_chunk[k]:
                                nc.tensor.matmul(
                                    ps[:, w0:w0 + W],
                                    g[:, lb * ATTR_DIM:(lb + 1) * ATTR_DIM],
                                    csel[:, cc:cc + W],
                                    start=False, stop=False)
                    nc.tensor.matmul(ps[:, :nsl], zrow[:], zcols[0:1, :nsl],
                                     start=False, stop=True)
                    nc.scalar.copy(sTbf[:, s0:s0 + nsl], ps[:, :nsl])
            if "phaseC" in skip and "phaseD" not in skip:
                nc.vector.memset(sTbf[:], 0.0)

            # ---------------- phase D: dense matmuls + relu + store ------
            with tc.tile_pool(name="eppsum", bufs=4, space="PSUM") as eppp:
                for k0 in ([] if "phaseD" in skip
                           else [kk for _ in range(repeat)
                                 for kk in range(0, N_SLOT3, 128)]):
                    xa = workp.tile([128, 2, 128], bf16, tag="xa")
                    nc.sync.dma_start(
                        xa[:],
                        xt_in[:].rearrange("p (a b) -> p a b", a=2)[:, :, k0:k0 + 128])
                    po = eppp.tile([128, OUT_DIM], f32, tag="po")
                    nc.tensor.matmul(po[:], xa[:, 0, :], w1t0_sb[:],
                                     start=True, stop=False)
                    nc.tensor.matmul(po[:], xa[:, 1, :], w1t1_sb[:],
                                     start=False, stop=False)
                    nc.tensor.matmul(po[:], sTbf[:, k0:k0 + 128], wct_sb[:],
                                     start=False, stop=not has_bias)
                    if has_bias:
                        nc.tensor.matmul(po[:], bias_lhsT_sb[:, k0:k0 + 128],
                                         bias_rhs_sb[:], start=False, stop=True)
                    ob = workp.tile([128, OUT_DIM], f32, tag="ob")
                    nc.scalar.activation(ob[:], po[:],
                                         mybir.ActivationFunctionType.Relu)
                    nc.sync.dma_start(out[k0:k0 + 128, :], ob[:])

    nc.finalize()
    return nc


def kernel3(object_feats, attr_feats, edge_index, edge_weight,
            W_a2o, b_a2o, W_proj, b_proj, W_upd, b_upd):
    structure, plan, in_maps, perms = _prepare3(
        object_feats, attr_feats, edge_index, edge_weight,
        W_a2o, b_a2o, W_proj, b_proj, W_upd, b_upd)
    nc = _compiled_cache.get(structure)
    if nc is None:
        nc = _build3(*plan)
        _compiled_cache[structure] = nc
    res = run_bass_kernel_spmd(nc, in_maps, list(range(NCORES))).results
    out = np.empty((B, N_OBJ, OUT_DIM), dtype=np.float32)
    for c in range(NCORES):
        out[c][perms[c]] = res[c]["out"][:N_OBJ]
    return out


# ================================================================ entry point
def kernel(**inputs):
    """Main entry: v3 PE-matmul-segsum pipeline (HW-validated, rel err ~2.3e-3)."""
    return kernel3(**inputs)


def kernel_v1(object_feats, attr_feats, edge_index, edge_weight,
              W_a2o, b_a2o, W_proj, b_proj, W_upd, b_upd):
    structure, plan, in_maps, metas = _prepare(
        object_feats, attr_feats, edge_index, edge_weight,
        W_a2o, b_a2o, W_proj, b_proj, W_upd, b_upd)
    chunks, T, n_slot, has_bias = plan

    nc = _compiled_cache.get(structure)
    if nc is None:
        nc = _build(chunks, T, n_slot, has_bias)
        _compiled_cache[structure] = nc

    res = run_bass_kernel_spmd(nc, in_maps, list(range(NCORES))).results

    out = np.zeros((B, N_OBJ, OUT_DIM), dtype=np.float32)
    for c in range(NCORES):
        perm = metas[c]
        real = perm >= 0
        out[c][perm[real]] = res[c]["out"][real]
    return out



# revision 21
# speedup vs baseline: 2.2643x; 2.2643x over previous
"""Trainium2 Bass kernel for nn_BipartiteMessagePassingLayer.

Reference computation (per batch element, flattened over the batch):
    attr_msgs = flat_attrs[src_attr] @ W_a2o.T + b_a2o          # [E, 256]
    weighted  = attr_msgs * w[:, None]
    agg       = segsum(weighted, src_obj) / clip(segsum(w, src_obj), 1e-6)
    proj      = agg @ W_proj.T + b_proj
    out       = relu(concat([X, proj]) @ W_upd.T + b_upd)

Algebraic restructuring used here (exact up to float reassociation):
    Let W1 = W_upd[:, :256], W2 = W_upd[:, 256:]
    Let Wc = W2 @ W_proj @ W_a2o        (256 x 128)
    Let s_o = sum_e  (w_e / c_o) * flat_attrs[src_attr_e]   over edges of o,
        where c_o = max(segsum(w)_o, 1e-6)   (weights normalized host-side)
    out = relu( X @ W1.T + s @ Wc.T + r1_o * (W2@W_proj@b_a2o) + (W2@b_proj + b_upd) )
        with r1_o = segsum(w)_o / c_o  (0 or 1 for all practical inputs).

Sharding: data-parallel over batch. Core c owns objects of batch c; edges are
bucketed to the core owning their src_obj.  The full attr table (16000 x 128,
bf16) is replicated into SBUF of every core; per-edge attr rows are fetched
with the SBUF-source transposed dma_gather, weighted via a PE outer-product
broadcast of the edge weights + DVE multiply, and segment-summed with a
strided DVE tensor_reduce over host-built fixed-capacity degree buckets.
Dense work (X @ W1.T + s @ Wc.T, relu) runs on the PE in bf16.
"""

import os

os.environ.setdefault("JAX_PLATFORMS", "")

import numpy as np
import ml_dtypes

import concourse.bass as bass
import concourse.tile as tile
from concourse import bacc, mybir
from concourse.bass_utils import run_bass_kernel_spmd
from concourse.masks import make_identity

# ---------------------------------------------------------------- constants
B, N_OBJ, N_ATTR = 8, 10000, 2000
IN_DIM, OUT_DIM, ATTR_DIM = 256, 256, 128
E_TOT = 800000
NCORES = 8
NA_TOT = B * N_ATTR  # 16000 attr rows, gathered globally

CHUNK_TOK = 2048  # tokens per gather/mult chunk (psum: 2048 f32 = 4 banks)
R_BASE = [4, 8, 12, 16, 24, 32, 64]  # degree-bucket capacities

_BF16 = ml_dtypes.bfloat16

_compiled_cache: dict = {}


# ================================================================ host prep
def _bucket_structure(degs_per_core):
    """Pick a common (all-cores) bucket structure.

    Returns (r_list, n_per_bucket) with n_per_bucket counts that are >= the
    per-core needs, and sum(n) a multiple of 128."""
    max_deg = max(int(d.max()) for d in degs_per_core)
    r_list = list(R_BASE)
    while r_list[-1] < max_deg:
        r_list.append(r_list[-1] * 2)
    r_arr = np.asarray(r_list)
    counts = np.zeros(len(r_list), dtype=np.int64)
    for d in degs_per_core:
        b = np.searchsorted(r_arr, np.maximum(d, 1))
        counts = np.maximum(counts, np.bincount(b, minlength=len(r_list)))
    counts = counts + 16  # per-bucket slack
    n_slot = int(counts.sum())
    pad = (-n_slot) % 128
    counts[0] += pad  # pad N_SLOT to a multiple of 128 with tiny-bucket slots
    return r_list, [int(c) for c in counts]


def _chunk_plan(r_list, n_per_bucket):
    """Static chunk list shared by all cores.

    Each chunk: (tok0, ntok, nobj, R, slot0).  ntok is a multiple of 128,
    whole objects only (nobj*R <= ntok)."""
    chunks = []
    tok0 = 0
    slot0 = 0
    for R, n in zip(r_list, n_per_bucket):
        opc = CHUNK_TOK // R  # objects per full chunk
        i = 0
        while i < n:
            nobj = min(opc, n - i)
            ntok = -(-nobj * R // 128) * 128  # round up to 128
            chunks.append((tok0, ntok, nobj, R, slot0 + i))
            tok0 += ntok
            i += nobj
        slot0 += n
    t_total = tok0
    n_slot = slot0
    assert n_slot == sum(n_per_bucket)
    return chunks, t_total, n_slot


def _prepare(object_feats, attr_feats, edge_index, edge_weight,
             W_a2o, b_a2o, W_proj, b_proj, W_upd, b_upd):
    """All host-side sharding/packing. Returns (structure, in_maps, meta)."""
    src_obj = np.asarray(edge_index[0]).astype(np.int64)
    src_attr = np.asarray(edge_index[1]).astype(np.int64)
    w = np.asarray(edge_weight, dtype=np.float32)
    X = np.asarray(object_feats, dtype=np.float32)
    A = np.asarray(attr_feats, dtype=np.float32).reshape(NA_TOT, ATTR_DIM)

    core_of = src_obj // N_OBJ
    per_core = []
    degs = []
    for c in range(NCORES):
        m = core_of == c
        lobj = (src_obj[m] - c * N_OBJ).astype(np.int64)
        per_core.append((lobj, src_attr[m].astype(np.int64), w[m]))
        degs.append(np.bincount(lobj, minlength=N_OBJ))

    r_list, n_per_bucket = _bucket_structure(degs)
    chunks, T, n_slot = _chunk_plan(r_list, n_per_bucket)
    r_arr = np.asarray(r_list)

    # ---- weights (host factoring; exact in f64 then cast) ----
    W_upd64 = np.asarray(W_upd, dtype=np.float64)
    W1 = W_upd64[:, :IN_DIM]
    W2 = W_upd64[:, IN_DIM:]
    Wp = np.asarray(W_proj, dtype=np.float64)
    Wa = np.asarray(W_a2o, dtype=np.float64)
    Wc = W2 @ Wp @ Wa  # [256, 128]
    ba2 = W2 @ Wp @ np.asarray(b_a2o, dtype=np.float64)
    bconst = W2 @ np.asarray(b_proj, dtype=np.float64) + np.asarray(b_upd, np.float64)
    has_bias = bool(np.any(ba2 != 0.0) or np.any(bconst != 0.0))

    w1t_bf = np.ascontiguousarray(
        W1.T.reshape(2, 128, OUT_DIM).astype(_BF16))        # [2,128,256]
    wct_bf = np.ascontiguousarray(Wc.T.astype(_BF16))       # [128,256]
    bias_rhs = np.stack([ba2, bconst]).astype(_BF16)        # [2,256]

    # attr table rearranged so row i sits at partition i%128, rank i//128
    attr_re = np.ascontiguousarray(
        A.reshape(NA_TOT // 128, 128, ATTR_DIM).transpose(1, 0, 2)
        .reshape(128, (NA_TOT // 128) * ATTR_DIM).astype(_BF16))

    in_maps = []
    metas = []
    for c in range(NCORES):
        lobj, lattr, wv = per_core[c]
        deg = degs[c]
        ws = np.zeros(N_OBJ, dtype=np.float64)
        np.add.at(ws, lobj, wv.astype(np.float64))
        assert not np.any((ws > 0) & (ws <= 2e-6)), "r1 edge case hit"
        cdenom = np.maximum(ws, 1e-6)
        wprime = (wv / cdenom[lobj]).astype(np.float32)

        # slot assignment: bucket-major, real objects then dummies
        bucket_of = np.searchsorted(r_arr, np.maximum(deg, 1))
        perm = np.full(n_slot, -1, dtype=np.int64)  # slot -> local obj id
        slot_base = 0
        obj_slot = np.empty(N_OBJ, dtype=np.int64)
        for bi, n in enumerate(n_per_bucket):
            objs = np.nonzero(bucket_of == bi)[0]
            assert len(objs) <= n, (bi, len(objs), n)
            perm[slot_base:slot_base + len(objs)] = objs
            obj_slot[objs] = slot_base + np.arange(len(objs))
            slot_base += n

        # token stream: edges sorted by slot; per-object tokens padded to R
        order = np.argsort(obj_slot[lobj], kind="stable")
        s_lattr = lattr[order]
        s_wp = wprime[order]
        s_slot = obj_slot[lobj][order]

        tok_attr = np.zeros(T, dtype=np.int16)
        tok_w = np.zeros(T, dtype=np.float32)
        # per-slot degree (0 for dummies)
        slot_deg = np.zeros(n_slot, dtype=np.int64)
        slot_deg[obj_slot] = deg
        # destination token index for each sorted edge:
        # chunk-level base for the slot + within-object rank
        slot_tok0 = np.zeros(n_slot, dtype=np.int64)
        for (tok0, ntok, nobj, R, sl0) in chunks:
            slot_tok0[sl0:sl0 + nobj] = tok0 + np.arange(nobj) * R
        within = np.zeros(len(order), dtype=np.int64)
        if len(order):
            boundaries = np.nonzero(np.diff(s_slot))[0] + 1
            starts = np.zeros(len(order), dtype=np.int64)
            starts[boundaries] = boundaries
            starts = np.maximum.accumulate(starts)
            within = np.arange(len(order)) - starts
        dst = slot_tok0[s_slot] + within
        tok_attr[dst] = s_lattr.astype(np.int16)
        tok_w[dst] = s_wp

        # wrapped [16, T/16], replicated to all 8 GPSIMD core groups -> [128, T/16]
        widx16 = tok_attr.reshape(T // 16, 16).T
        widx = np.ascontiguousarray(np.tile(widx16, (8, 1)))
        wtok = tok_w.astype(_BF16)                     # [T] bf16

        xperm = np.zeros((n_slot, IN_DIM), dtype=np.float32)
        real = perm >= 0
        xperm[real] = X[c][perm[real]]

        im = {
            "attr_re": attr_re,
            "xperm": xperm,
            "widx": widx,
            "wtok": wtok,
            "w1t0": w1t_bf[0],
            "w1t1": w1t_bf[1],
            "wct": wct_bf,
        }
        if has_bias:
            ind = np.zeros((2, n_slot), dtype=_BF16)
            ind[0, :] = ((ws > 1e-6)[perm.clip(0)] & real).astype(_BF16)
            ind[1, :] = real.astype(_BF16)
            im["bias_lhsT"] = ind
            im["bias_rhs"] = bias_rhs
        in_maps.append(im)
        metas.append(perm)

    structure = (tuple(r_list), tuple(n_per_bucket), has_bias)
    return structure, (chunks, T, n_slot, has_bias), in_maps, metas


# ================================================================ bass build
def _build(chunks, T, n_slot, has_bias, skip=()):
    nc = bacc.Bacc(None, target_bir_lowering=False, debug=False)
    f32 = mybir.dt.float32
    bf16 = mybir.dt.bfloat16

    attr_re = nc.declare_dram_parameter(
        "attr_re", [128, (NA_TOT // 128) * ATTR_DIM], bf16, isOutput=False)
    xperm = nc.declare_dram_parameter("xperm", [n_slot, IN_DIM], f32, isOutput=False)
    widx = nc.declare_dram_parameter("widx", [128, T // 16], mybir.dt.int16, isOutput=False)
    wtok = nc.declare_dram_parameter("wtok", [T], bf16, isOutput=False)
    w1t0 = nc.declare_dram_parameter("w1t0", [128, OUT_DIM], bf16, isOutput=False)
    w1t1 = nc.declare_dram_parameter("w1t1", [128, OUT_DIM], bf16, isOutput=False)
    wct = nc.declare_dram_parameter("wct", [ATTR_DIM, OUT_DIM], bf16, isOutput=False)
    if has_bias:
        bias_lhsT = nc.declare_dram_parameter("bias_lhsT", [2, n_slot], bf16, isOutput=False)
        bias_rhs = nc.declare_dram_parameter("bias_rhs", [2, OUT_DIM], bf16, isOutput=False)
    out = nc.declare_dram_parameter("out", [n_slot, OUT_DIM], f32, isOutput=True)

    n_ranks = NA_TOT // 128  # 125

    with tile.TileContext(nc) as tc:
        with (
            tc.tile_pool(name="const", bufs=1) as constp,
            tc.tile_pool(name="big", bufs=1) as bigp,
            tc.tile_pool(name="work", bufs=3) as workp,
        ):
            # ---------------- persistent SBUF tensors
            table = constp.tile([128, NA_TOT], bf16, tag="table")
            xT = bigp.tile([128, 2, n_slot], bf16, tag="xT")
            sT = bigp.tile([128, n_slot], f32, tag="sT")
            sTbf = bigp.tile([128, n_slot], bf16, tag="sTbf")
            ident = constp.tile([128, 128], bf16, tag="ident")
            ones_row = constp.tile([1, 128], bf16, tag="ones")
            w1t0_sb = constp.tile([128, OUT_DIM], bf16, tag="w1t0")
            w1t1_sb = constp.tile([128, OUT_DIM], bf16, tag="w1t1")
            wct_sb = constp.tile([128, OUT_DIM], bf16, tag="wct")
            make_identity(nc, ident[:])
            nc.gpsimd.memset(ones_row[:], 1.0)
            nc.sync.dma_start(w1t0_sb[:], w1t0[:])
            nc.sync.dma_start(w1t1_sb[:], w1t1[:])
            nc.sync.dma_start(wct_sb[:], wct[:])
            if has_bias:
                bias_lhsT_sb = constp.tile([2, n_slot], bf16, tag="biasl")
                bias_rhs_sb = constp.tile([2, OUT_DIM], bf16, tag="biasr")
                nc.sync.dma_start(bias_lhsT_sb[:], bias_lhsT[:])
                nc.sync.dma_start(bias_rhs_sb[:], bias_rhs[:])

            # ---------------- phase A: attr table load (host already bf16)
            nc.sync.dma_start(table[:], attr_re[:])

            # ---------------- phase B: X -> bf16 -> transpose -> xT
            with tc.tile_pool(name="xpsum", bufs=2, space="PSUM") as xpp:
                for k0 in ([] if "phaseB" in skip else range(0, n_slot, 128)):
                    xf = workp.tile([128, IN_DIM], f32, tag="xf")
                    xb = workp.tile([128, IN_DIM], bf16, tag="xb")
                    nc.sync.dma_start(xf[:], xperm[k0:k0 + 128, :])
                    nc.scalar.copy(xb[:], xf[:])
                    for j in range(2):
                        pt = xpp.tile([128, 128], bf16, tag="xtp")
                        nc.tensor.transpose(pt[:], xb[:, j * 128:(j + 1) * 128], ident[:])
                        nc.scalar.copy(xT[:, j, k0:k0 + 128], pt[:])

            # ---------------- phase C: gather + weight + segmented reduce
            with tc.tile_pool(name="wps", bufs=2, space="PSUM") as wpsp:
                for (tok0, ntok, nobj, R, slot0) in ([] if "phaseC" in skip else chunks):
                    idxt = workp.tile([128, CHUNK_TOK // 16], mybir.dt.int16, tag="idx")
                    nc.sync.dma_start(idxt[:, :ntok // 16],
                                      widx[:, tok0 // 16:(tok0 + ntok) // 16])
                    g = workp.tile([128, CHUNK_TOK], bf16, tag="g")
                    if "gather" not in skip:
                        nc.gpsimd.dma_gather(
                            g[:, :ntok].rearrange("p (one t) -> p one t", one=1),
                            table[:],
                            idxt[:, :ntok // 16],
                            num_idxs=ntok,
                            num_idxs_reg=ntok,
                            elem_size=ATTR_DIM,
                            transpose=True,
                            sbuf_tokens_per_rank=128,
                            sbuf_free_dim_per_rank=ATTR_DIM * 2,
                            single_packet=False,
                        )
                    wrow = workp.tile([1, CHUNK_TOK], bf16, tag="wrow")
                    nc.sync.dma_start(wrow[0:1, :ntok], wtok[None, tok0:tok0 + ntok])
                    wps = wpsp.tile([128, CHUNK_TOK], f32, tag="wps")
                    if "wrep" not in skip:
                        for j0 in range(0, ntok, 512):
                            n = min(512, ntok - j0)
                            nc.tensor.matmul(
                                wps[:, j0:j0 + n], ones_row[:],
                                wrow[0:1, j0:j0 + n], start=True, stop=True)
                    wx = workp.tile([128, CHUNK_TOK], bf16, tag="wx")
                    if "mult" not in skip:
                        nc.vector.tensor_tensor(
                            wx[:, :ntok], g[:, :ntok], wps[:, :ntok],
                            mybir.AluOpType.mult)
                    if "reduce" not in skip:
                        nc.vector.tensor_reduce(
                            sT[:, slot0:slot0 + nobj],
                            wx[:, :nobj * R].rearrange("p (o r) -> p o r", r=R),
                            axis=mybir.AxisListType.X,
                            op=mybir.AluOpType.add)

            # bulk convert sT -> bf16 for the dense matmul
            if not ({"phaseC", "reduce", "phaseD"} & set(skip)):
                nc.scalar.copy(sTbf[:], sT[:])

            # ---------------- phase D: dense matmuls + relu + store
            with tc.tile_pool(name="eppsum", bufs=2, space="PSUM") as eppp:
                for k0 in ([] if ("phaseD" in skip or "phaseB" in skip or
                                  "phaseC" in skip or "reduce" in skip)
                           else range(0, n_slot, 128)):
                    po = eppp.tile([128, OUT_DIM], f32, tag="po")
                    nc.tensor.matmul(po[:], xT[:, 0, k0:k0 + 128], w1t0_sb[:],
                                     start=True, stop=False)
                    nc.tensor.matmul(po[:], xT[:, 1, k0:k0 + 128], w1t1_sb[:],
                                     start=False, stop=False)
                    nc.tensor.matmul(po[:], sTbf[:, k0:k0 + 128], wct_sb[:],
                                     start=False, stop=not has_bias)
                    if has_bias:
                        nc.tensor.matmul(po[:], bias_lhsT_sb[:, k0:k0 + 128],
                                         bias_rhs_sb[:], start=False, stop=True)
                    ob = workp.tile([128, OUT_DIM], f32, tag="ob")
                    nc.scalar.activation(ob[:], po[:],
                                         mybir.ActivationFunctionType.Relu)
                    nc.sync.dma_start(out[k0:k0 + 128, :], ob[:])

    nc.finalize()
    return nc


# ================================================================ v2: scan-based
OBJ_CHUNK = 512          # objects per chunk (20 chunks cover 10240 slots)
N_SLOT2 = 10240
SUB_TOK = 2048           # sub-chunk size for W_rep psum / mult / scan
DMA_GATHER_FRAC = 1, 1   # route chunk k to dma_gather if (k*a)%b < a


def _weights_prep(W_a2o, b_a2o, W_proj, b_proj, W_upd, b_upd):
    W_upd64 = np.asarray(W_upd, dtype=np.float64)
    W1 = W_upd64[:, :IN_DIM]
    W2 = W_upd64[:, IN_DIM:]
    Wp = np.asarray(W_proj, dtype=np.float64)
    Wa = np.asarray(W_a2o, dtype=np.float64)
    Wc = W2 @ Wp @ Wa
    ba2 = W2 @ Wp @ np.asarray(b_a2o, dtype=np.float64)
    bconst = W2 @ np.asarray(b_proj, dtype=np.float64) + np.asarray(b_upd, np.float64)
    has_bias = bool(np.any(ba2 != 0.0) or np.any(bconst != 0.0))
    w1t_bf = np.ascontiguousarray(W1.T.reshape(2, 128, OUT_DIM).astype(_BF16))
    wct_bf = np.ascontiguousarray(Wc.T.astype(_BF16))
    bias_rhs = np.stack([ba2, bconst]).astype(_BF16)
    return w1t_bf, wct_bf, bias_rhs, has_bias


def _prepare2(object_feats, attr_feats, edge_index, edge_weight,
              W_a2o, b_a2o, W_proj, b_proj, W_upd, b_upd):
    src_obj = np.asarray(edge_index[0]).astype(np.int64)
    src_attr = np.asarray(edge_index[1]).astype(np.int64)
    w = np.asarray(edge_weight, dtype=np.float32)
    X = np.asarray(object_feats, dtype=np.float32)
    A = np.asarray(attr_feats, dtype=np.float32).reshape(NA_TOT, ATTR_DIM)

    w1t_bf, wct_bf, bias_rhs, has_bias = _weights_prep(
        W_a2o, b_a2o, W_proj, b_proj, W_upd, b_upd)

    core_of = src_obj // N_OBJ
    per_core = []
    for c in range(NCORES):
        m = core_of == c
        lobj = (src_obj[m] - c * N_OBJ).astype(np.int64)
        per_core.append((lobj, src_attr[m].astype(np.int64), w[m]))

    nchunks = N_SLOT2 // OBJ_CHUNK
    # shared static token capacity per chunk
    cap = 0
    degs = []
    for c in range(NCORES):
        deg = np.bincount(per_core[c][0], minlength=N_SLOT2)
        degs.append(deg)
        per_chunk = deg.reshape(nchunks, OBJ_CHUNK).sum(axis=1)
        cap = max(cap, int(per_chunk.max()))
    cap = -(-(cap) // 128) * 128
    T = nchunks * cap

    # row-wrap layout for dma_gather (row i at partition i%128, rank i//128)
    attr_re = np.ascontiguousarray(
        A.reshape(NA_TOT // 128, 128, ATTR_DIM).transpose(1, 0, 2)
        .reshape(128, (NA_TOT // 128) * ATTR_DIM).astype(_BF16))
    # transposed layout for indirect_copy (dim on partitions)
    attr_t = np.ascontiguousarray(A.T.astype(_BF16))

    in_maps = []
    for c in range(NCORES):
        lobj, lattr, wv = per_core[c]
        deg = degs[c]
        ws = np.zeros(N_SLOT2, dtype=np.float64)
        np.add.at(ws, lobj, wv.astype(np.float64))
        assert not np.any((ws > 0) & (ws <= 2e-6)), "r1 edge case hit"
        wprime = (wv / np.maximum(ws, 1e-6)[lobj]).astype(np.float32)

        order = np.argsort(lobj, kind="stable")
        s_attr = lattr[order]
        s_w = wprime[order]
        cum = np.concatenate([[0], np.cumsum(deg)])

        tok_attr = np.zeros(T, dtype=np.int16)
        tok_w = np.zeros(T, dtype=np.float32)
        bends = np.zeros((nchunks, OBJ_CHUNK), dtype=np.int16)
        for k in range(nchunks):
            o0 = k * OBJ_CHUNK
            e0, e1 = cum[o0], cum[o0 + OBJ_CHUNK]
            n = e1 - e0
            assert n <= cap
            tok_attr[k * cap:k * cap + n] = s_attr[e0:e1]
            tok_w[k * cap:k * cap + n] = s_w[e0:e1]
            bends[k] = (cum[o0 + 1:o0 + OBJ_CHUNK + 1] - e0).astype(np.int16)

        widx = np.ascontiguousarray(
            np.tile(tok_attr.reshape(T // 16, 16).T, (8, 1)))
        bidx = np.ascontiguousarray(
            np.tile(bends.reshape(nchunks * OBJ_CHUNK // 16, 16).T, (8, 1)))
        wtok = tok_w.astype(_BF16)

        xt = np.zeros((128, 2, N_SLOT2), dtype=_BF16)
        xt[:, 0, :N_OBJ] = X[c][:, :128].T
        xt[:, 1, :N_OBJ] = X[c][:, 128:].T

        im = {
            "attr_re": attr_re,
            "attr_t": attr_t,
            "xt": np.ascontiguousarray(xt.reshape(128, 2 * N_SLOT2)),
            "widx": widx,
            "bidx": bidx,
            "wtok": wtok,
            "w1t0": w1t_bf[0],
            "w1t1": w1t_bf[1],
            "wct": wct_bf,
        }
        if has_bias:
            ind = np.zeros((2, N_SLOT2), dtype=_BF16)
            ind[0, :] = (ws > 1e-6).astype(_BF16)
            ind[1, :N_OBJ] = 1.0
            im["bias_lhsT"] = ind
            im["bias_rhs"] = bias_rhs
        in_maps.append(im)

    structure = ("v2", cap, has_bias)
    return structure, (cap, T, has_bias), in_maps


def _build2(cap, T, has_bias, dma_frac=DMA_GATHER_FRAC, repeat=1,
            wrep_evict=False, relu_dve=False, nqueues=1, skip=()):
    nc = bacc.Bacc(None, target_bir_lowering=False, debug=False,
                   num_swdge_queues=nqueues)
    f32 = mybir.dt.float32
    bf16 = mybir.dt.bfloat16
    i16 = mybir.dt.int16
    nchunks = N_SLOT2 // OBJ_CHUNK

    attr_re = nc.declare_dram_parameter(
        "attr_re", [128, (NA_TOT // 128) * ATTR_DIM], bf16, isOutput=False)
    attr_t = nc.declare_dram_parameter("attr_t", [128, NA_TOT], bf16, isOutput=False)
    xt_in = nc.declare_dram_parameter("xt", [128, 2 * N_SLOT2], bf16, isOutput=False)
    widx = nc.declare_dram_parameter("widx", [128, T // 16], i16, isOutput=False)
    bidx = nc.declare_dram_parameter("bidx", [128, N_SLOT2 // 16], i16, isOutput=False)
    wtok = nc.declare_dram_parameter("wtok", [T], bf16, isOutput=False)
    w1t0 = nc.declare_dram_parameter("w1t0", [128, OUT_DIM], bf16, isOutput=False)
    w1t1 = nc.declare_dram_parameter("w1t1", [128, OUT_DIM], bf16, isOutput=False)
    wct = nc.declare_dram_parameter("wct", [ATTR_DIM, OUT_DIM], bf16, isOutput=False)
    if has_bias:
        bias_lhsT = nc.declare_dram_parameter("bias_lhsT", [2, N_SLOT2], bf16, isOutput=False)
        bias_rhs = nc.declare_dram_parameter("bias_rhs", [2, OUT_DIM], bf16, isOutput=False)
    out = nc.declare_dram_parameter("out", [N_SLOT2, OUT_DIM], f32, isOutput=True)

    with tile.TileContext(nc) as tc:
        with (
            tc.tile_pool(name="const", bufs=1) as constp,
            tc.tile_pool(name="big", bufs=1) as bigp,
            tc.tile_pool(name="work", bufs=3) as workp,
            tc.tile_pool(name="gp", bufs=2) as gpool,
            tc.tile_pool(name="pfx", bufs=1) as pfxp,
        ):
            use_pool_gather = dma_frac[0] < dma_frac[1]
            table_r = constp.tile([128, NA_TOT], bf16, tag="table_r")
            table_t = (constp.tile([128, NA_TOT], bf16, tag="table_t")
                       if use_pool_gather else None)
            sTbf = bigp.tile([128, N_SLOT2], bf16, tag="sTbf")
            ones_row = constp.tile([1, 128], bf16, tag="ones")
            w1t0_sb = constp.tile([128, OUT_DIM], bf16, tag="w1t0")
            w1t1_sb = constp.tile([128, OUT_DIM], bf16, tag="w1t1")
            wct_sb = constp.tile([128, OUT_DIM], bf16, tag="wct")
            bidx_sb = constp.tile([128, N_SLOT2 // 16], i16, tag="bidx")
            nc.gpsimd.memset(ones_row[:], 1.0)
            nc.sync.dma_start(table_r[:], attr_re[:])
            if use_pool_gather:
                nc.sync.dma_start(table_t[:], attr_t[:])
            nc.sync.dma_start(w1t0_sb[:], w1t0[:])
            nc.sync.dma_start(w1t1_sb[:], w1t1[:])
            nc.sync.dma_start(wct_sb[:], wct[:])
            nc.sync.dma_start(bidx_sb[:], bidx[:])
            if has_bias:
                bias_lhsT_sb = constp.tile([2, N_SLOT2], bf16, tag="biasl")
                bias_rhs_sb = constp.tile([2, OUT_DIM], bf16, tag="biasr")
                nc.sync.dma_start(bias_lhsT_sb[:], bias_lhsT[:])
                nc.sync.dma_start(bias_rhs_sb[:], bias_rhs[:])

            zero_col = constp.tile([128, 1], f32, tag="zcol")
            nc.vector.memset(zero_col[:], 0.0)
            if ("phaseC" in skip or "subtract" in skip) and "phaseD" not in skip:
                nc.vector.memset(sTbf[:], 0.0)

            # ---------------- phase C: gather + weight + scan + boundary diff
            with tc.tile_pool(name="wps", bufs=2, space="PSUM") as wpsp:
                for k in [kk for _ in range(0 if "phaseC" in skip else repeat)
                          for kk in range(nchunks)]:
                    tok0 = k * cap
                    use_dma = (k * dma_frac[0]) % dma_frac[1] < dma_frac[0]
                    g = gpool.tile([128, cap], bf16, tag="g")
                    if "gather" in skip:
                        pass
                    elif use_dma:
                        idxt = workp.tile([128, cap // 16], i16, tag="idx")
                        nc.sync.dma_start(idxt[:],
                                          widx[:, tok0 // 16:(tok0 + cap) // 16])
                        nc.gpsimd.dma_gather(
                            g[:].rearrange("p (one t) -> p one t", one=1),
                            table_r[:],
                            idxt[:],
                            num_idxs=cap,
                            num_idxs_reg=cap,
                            elem_size=ATTR_DIM,
                            transpose=True,
                            sbuf_tokens_per_rank=128,
                            sbuf_free_dim_per_rank=ATTR_DIM * 2,
                            single_packet=False,
                            queue_num=k % nqueues,
                        )
                    else:
                        idxt = workp.tile([128, cap // 16], i16, tag="idx")
                        nc.sync.dma_start(idxt[:],
                                          widx[:, tok0 // 16:(tok0 + cap) // 16])
                        for j0 in range(0, cap, SUB_TOK):
                            n = min(SUB_TOK, cap - j0)
                            nc.gpsimd.indirect_copy(
                                g[:, j0:j0 + n], table_t[:],
                                idxt[:, j0 // 16:(j0 + n) // 16].bitcast(mybir.dt.uint16),
                                i_know_ap_gather_is_preferred=True)

                    # exclusive prefix: prefix[:, j] = sum of tokens < j
                    prefix = pfxp.tile([128, 1 + cap], f32, tag="prefix")
                    nc.vector.memset(prefix[:, 0:1], 0.0)
                    for j0 in range(0, cap, SUB_TOK):
                        n = min(SUB_TOK, cap - j0)
                        wrow = workp.tile([1, SUB_TOK], bf16, tag="wrow")
                        nc.sync.dma_start(wrow[0:1, :n],
                                          wtok[None, tok0 + j0:tok0 + j0 + n])
                        wps = wpsp.tile([128, SUB_TOK], f32, tag="wps")
                        if "wrep" not in skip:
                            for i0 in range(0, n, 512):
                                m = min(512, n - i0)
                                nc.tensor.matmul(
                                    wps[:, i0:i0 + m], ones_row[:],
                                    wrow[0:1, i0:i0 + m], start=True, stop=True)
                        wx = workp.tile([128, SUB_TOK], bf16, tag="wx")
                        if "mult" in skip:
                            pass
                        elif wrep_evict:
                            wrb = workp.tile([128, SUB_TOK], bf16, tag="wrb")
                            nc.scalar.copy(wrb[:, :n], wps[:, :n])
                            nc.vector.scalar_tensor_tensor(
                                out=wx[:, :n], in0=g[:, j0:j0 + n], scalar=1.0,
                                in1=wrb[:, :n], op0=mybir.AluOpType.mult,
                                op1=mybir.AluOpType.mult)
                        else:
                            nc.vector.scalar_tensor_tensor(
                                out=wx[:, :n], in0=g[:, j0:j0 + n], scalar=1.0,
                                in1=wps[:, :n], op0=mybir.AluOpType.mult,
                                op1=mybir.AluOpType.mult)
                        if "scan" not in skip:
                            init = 0.0 if j0 == 0 else prefix[:, j0:j0 + 1]
                            nc.vector.tensor_tensor_scan(
                                prefix[:, 1 + j0:1 + j0 + n], wx[:, :n],
                                zero_col[:].to_broadcast([128, n]),
                                initial=init,
                                op0=mybir.AluOpType.add, op1=mybir.AluOpType.add)

                    gbuf = workp.tile([128, OBJ_CHUNK + 16], f32, tag="gbuf")
                    nc.vector.memset(gbuf[:, 0:1], 0.0)
                    if "apgather" not in skip:
                        nc.gpsimd.ap_gather(
                            gbuf[:, 1:1 + OBJ_CHUNK].rearrange("p (n one) -> p n one", one=1),
                            prefix[:].rearrange("p (n one) -> p n one", one=1),
                            bidx_sb[:, k * OBJ_CHUNK // 16:(k + 1) * OBJ_CHUNK // 16],
                            channels=128, num_elems=1 + cap, d=1, num_idxs=OBJ_CHUNK)
                    if "subtract" not in skip:
                        nc.vector.tensor_tensor(
                            sTbf[:, k * OBJ_CHUNK:(k + 1) * OBJ_CHUNK],
                            gbuf[:, 1:1 + OBJ_CHUNK], gbuf[:, 0:OBJ_CHUNK],
                            mybir.AluOpType.subtract)

            # ---------------- phase D: dense matmuls + relu + store
            with tc.tile_pool(name="eppsum", bufs=4, space="PSUM") as eppp:
                for k0 in [kk for _ in range(0 if "phaseD" in skip else repeat)
                           for kk in range(0, N_SLOT2, 128)]:
                    xa = workp.tile([128, 2, 128], bf16, tag="xa")
                    nc.sync.dma_start(
                        xa[:],
                        xt_in[:].rearrange("p (a b) -> p a b", a=2)[:, :, k0:k0 + 128])
                    po = eppp.tile([128, OUT_DIM], f32, tag="po")
                    nc.tensor.matmul(po[:], xa[:, 0, :], w1t0_sb[:],
                                     start=True, stop=False)
                    nc.tensor.matmul(po[:], xa[:, 1, :], w1t1_sb[:],
                                     start=False, stop=False)
                    nc.tensor.matmul(po[:], sTbf[:, k0:k0 + 128], wct_sb[:],
                                     start=False, stop=not has_bias)
                    if has_bias:
                        nc.tensor.matmul(po[:], bias_lhsT_sb[:, k0:k0 + 128],
                                         bias_rhs_sb[:], start=False, stop=True)
                    ob = workp.tile([128, OUT_DIM], f32, tag="ob")
                    if relu_dve:
                        nc.vector.tensor_scalar_max(ob[:], po[:], 0.0)
                    else:
                        nc.scalar.activation(ob[:], po[:],
                                             mybir.ActivationFunctionType.Relu)
                    nc.sync.dma_start(out[k0:k0 + 128, :], ob[:])

    nc.finalize()
    return nc


def kernel2(object_feats, attr_feats, edge_index, edge_weight,
            W_a2o, b_a2o, W_proj, b_proj, W_upd, b_upd):
    structure, plan, in_maps = _prepare2(
        object_feats, attr_feats, edge_index, edge_weight,
        W_a2o, b_a2o, W_proj, b_proj, W_upd, b_upd)
    cap, T, has_bias = plan
    nc = _compiled_cache.get(structure)
    if nc is None:
        nc = _build2(cap, T, has_bias)
        _compiled_cache[structure] = nc
    res = run_bass_kernel_spmd(nc, in_maps, list(range(NCORES))).results
    out = np.stack([res[c]["out"][:N_OBJ] for c in range(NCORES)])
    return np.ascontiguousarray(out).reshape(B, N_OBJ, OUT_DIM)


# ================================================================ v3: PE-matmul segsum
# Layout: per-core objects degree-sorted into slots; slot capacities are the
# elementwise max of the 8 cores' sorted degree profiles (static, data-derived,
# shared by all cores).  Tokens (edges, padded to capacity) stream in slot
# order; each 128-token block is gathered in natural layout ([token, attr]) by
# a DRAM-source dma_gather and reduced into per-slot sums by a single PE
# matmul against a host-built selection matrix sel[token, slot_window] that
# carries the normalized edge weights.  No DVE/Pool work in the inner loop.
N_SLOT3 = 10112          # 10000 real slots + pad to a multiple of 128
TOKCAP3 = 6144           # max tokens per chunk (psum window: <=512 slots)


def _prepare3(object_feats, attr_feats, edge_index, edge_weight,
              W_a2o, b_a2o, W_proj, b_proj, W_upd, b_upd):
    src_obj = np.asarray(edge_index[0]).astype(np.int64)
    src_attr = np.asarray(edge_index[1]).astype(np.int64)
    w = np.asarray(edge_weight, dtype=np.float32)
    X = np.asarray(object_feats, dtype=np.float32)
    A = np.asarray(attr_feats, dtype=np.float32).reshape(NA_TOT, ATTR_DIM)

    w1t_bf, wct_bf, bias_rhs, has_bias = _weights_prep(
        W_a2o, b_a2o, W_proj, b_proj, W_upd, b_upd)

    core_of = src_obj // N_OBJ
    per_core = []
    degs = []
    perms = []
    for c in range(NCORES):
        m = core_of == c
        lobj = (src_obj[m] - c * N_OBJ).astype(np.int64)
        per_core.append((lobj, src_attr[m].astype(np.int64), w[m]))
        deg = np.bincount(lobj, minlength=N_OBJ)
        degs.append(deg)
        perms.append(np.argsort(-deg, kind="stable"))

    # ---- static slot-capacity profile (shared across cores) ----
    sorted_deg = np.stack([degs[c][perms[c]] for c in range(NCORES)])
    cap = sorted_deg.max(axis=0).astype(np.int64)
    capp = np.concatenate([cap, np.zeros(N_SLOT3 - N_OBJ, np.int64)])

    # ---- chunks: consecutive slots, <=512 slots and <=TOKCAP3 tokens ----
    chunks = []  # (s0, s1, tok0, ntok)
    s0 = 0
    tok0 = 0
    while s0 < N_SLOT3:
        s1 = s0
        t = 0
        while s1 < N_SLOT3 and s1 - s0 < 512 and t + capp[s1] <= TOKCAP3:
            t += int(capp[s1])
            s1 += 1
        ntok = -(-t // 128) * 128
        chunks.append((s0, s1, tok0, ntok))
        tok0 += ntok
        s0 = s1
    T = tok0

    # ---- static slot->token map and block windows ----
    slot_tok0 = np.zeros(N_SLOT3 + 1, dtype=np.int64)
    slot_of_tok = np.zeros(T, dtype=np.int64)
    for (s0, s1, tok0, ntok) in chunks:
        cs = np.concatenate([[0], np.cumsum(capp[s0:s1])])
        slot_tok0[s0:s1] = tok0 + cs[:-1]
        sl = np.repeat(np.arange(s0, s1), capp[s0:s1])
        sl = np.concatenate(
            [sl, np.full(ntok - len(sl), max(s1 - 1, s0), np.int64)])
        slot_of_tok[tok0:tok0 + ntok] = sl
    nblk = T // 128
    blk_first = slot_of_tok.reshape(nblk, 128)[:, 0]
    blk_last = slot_of_tok.reshape(nblk, 128)[:, -1]
    blk_W = (blk_last - blk_first + 1).astype(np.int64)
    colptr = np.concatenate([[0], np.cumsum(blk_W)]).astype(np.int64)
    ncols = int(colptr[-1])

    # per-chunk block lists: (local_block, psum_col0, W, chunk_col0)
    blk_of_chunk = []
    chunk_cols = []
    for (s0, s1, tok0, ntok) in chunks:
        b0 = tok0 // 128
        b1 = (tok0 + ntok) // 128
        cks = []
        for b in range(b0, b1):
            cks.append((b - b0, int(blk_first[b] - s0), int(blk_W[b]),
                        int(colptr[b] - colptr[b0])))
        blk_of_chunk.append(tuple(cks))
        chunk_cols.append((int(colptr[b0]), int(colptr[b1] - colptr[b0])))
    max_ck = max((c[1] for c in chunk_cols), default=1)

    attr_nat = np.ascontiguousarray(A.astype(_BF16))

    in_maps = []
    for c in range(NCORES):
        lobj, lattr, wv = per_core[c]
        deg = degs[c]
        perm = perms[c]
        ws = np.zeros(N_OBJ, dtype=np.float64)
        np.add.at(ws, lobj, wv.astype(np.float64))
        assert not np.any((ws > 0) & (ws <= 2e-6)), "r1 edge case hit"
        wprime = (wv / np.maximum(ws, 1e-6)[lobj]).astype(np.float32)

        slot_of_obj = np.empty(N_OBJ, dtype=np.int64)
        slot_of_obj[perm] = np.arange(N_OBJ)
        order = np.argsort(slot_of_obj[lobj], kind="stable")
        s_attr = lattr[order]
        s_w = wprime[order]
        s_slot = slot_of_obj[lobj][order]

        # within-slot rank of each sorted edge
        n = len(order)
        within = np.zeros(n, dtype=np.int64)
        if n:
            boundaries = np.nonzero(np.diff(s_slot))[0] + 1
            starts = np.zeros(n, dtype=np.int64)
            starts[boundaries] = boundaries
            starts = np.maximum.accumulate(starts)
            within = np.arange(n) - starts
        dst = slot_tok0[s_slot] + within

        tok_attr = np.zeros(T, dtype=np.int16)
        tok_attr[dst] = s_attr.astype(np.int16)
        widx = np.ascontiguousarray(
            np.tile(tok_attr.reshape(T // 16, 16).T, (8, 1)))

        sel = np.zeros((128, ncols), dtype=_BF16)
        blk = dst // 128
        part = dst % 128
        col = colptr[blk] + (s_slot - blk_first[blk])
        sel[part, col] = s_w

        xt = np.zeros((128, 2, N_SLOT3), dtype=_BF16)
        xt[:, 0, :N_OBJ] = X[c][perm, :128].T
        xt[:, 1, :N_OBJ] = X[c][perm, 128:].T

        im = {
            "attr_nat": attr_nat,
            "xt": np.ascontiguousarray(xt.reshape(128, 2 * N_SLOT3)),
            "widx": widx,
            "sel": sel,
            "w1t0": w1t_bf[0],
            "w1t1": w1t_bf[1],
            "wct": wct_bf,
        }
        if has_bias:
            ind = np.zeros((2, N_SLOT3), dtype=_BF16)
            ind[0, :N_OBJ] = (ws > 1e-6)[perm].astype(_BF16)
            ind[1, :N_OBJ] = 1.0
            im["bias_lhsT"] = ind
            im["bias_rhs"] = bias_rhs
        in_maps.append(im)

    structure = ("v3", T, ncols, has_bias, tuple(chunks),
                 tuple(blk_W.tolist()), tuple(blk_first.tolist()))
    plan = (tuple(chunks), blk_of_chunk, tuple(chunk_cols), max_ck, T, ncols,
            has_bias)
    return structure, plan, in_maps, perms


def _build3(chunks, blk_of_chunk, chunk_cols, max_ck, T, ncols, has_bias,
            nqueues=4, skip=(), repeat=1, piece=1024, single_packet=True,
            gbufs=3, store_grp=4):
    nc = bacc.Bacc(None, target_bir_lowering=False, debug=False,
                   num_swdge_queues=nqueues)
    f32 = mybir.dt.float32
    bf16 = mybir.dt.bfloat16
    i16 = mybir.dt.int16

    attr_nat = nc.declare_dram_parameter(
        "attr_nat", [NA_TOT, ATTR_DIM], bf16, isOutput=False)
    xt_in = nc.declare_dram_parameter("xt", [128, 2 * N_SLOT3], bf16, isOutput=False)
    widx = nc.declare_dram_parameter("widx", [128, T // 16], i16, isOutput=False)
    sel_in = nc.declare_dram_parameter("sel", [128, ncols], bf16, isOutput=False)
    w1t0 = nc.declare_dram_parameter("w1t0", [128, OUT_DIM], bf16, isOutput=False)
    w1t1 = nc.declare_dram_parameter("w1t1", [128, OUT_DIM], bf16, isOutput=False)
    wct = nc.declare_dram_parameter("wct", [ATTR_DIM, OUT_DIM], bf16, isOutput=False)
    if has_bias:
        bias_lhsT = nc.declare_dram_parameter("bias_lhsT", [2, N_SLOT3], bf16, isOutput=False)
        bias_rhs = nc.declare_dram_parameter("bias_rhs", [2, OUT_DIM], bf16, isOutput=False)
    out = nc.declare_dram_parameter("out", [N_SLOT3, OUT_DIM], bf16, isOutput=True)

    with tile.TileContext(nc) as tc:
        with (
            tc.tile_pool(name="const", bufs=1) as constp,
            tc.tile_pool(name="big", bufs=1) as bigp,
            tc.tile_pool(name="work", bufs=3) as workp,
            tc.tile_pool(name="gp", bufs=gbufs) as gpool,
        ):
            sTbf = bigp.tile([128, N_SLOT3], bf16, tag="sTbf")
            zrow = constp.tile([1, 128], bf16, tag="zrow")
            zcols = constp.tile([1, 512], bf16, tag="zcols")
            w1t0_sb = constp.tile([128, OUT_DIM], bf16, tag="w1t0")
            w1t1_sb = constp.tile([128, OUT_DIM], bf16, tag="w1t1")
            wct_sb = constp.tile([128, OUT_DIM], bf16, tag="wct")
            widx_sb = constp.tile([128, T // 16], i16, tag="widx")
            sel_sb = constp.tile([128, ncols], bf16, tag="sel")
            xt_sb = constp.tile([128, 2 * N_SLOT3], bf16, tag="xt")
            nc.gpsimd.memset(zrow[:], 0.0)
            nc.gpsimd.memset(zcols[:], 0.0)
            nc.sync.dma_start(widx_sb[:], widx[:])
            nc.sync.dma_start(sel_sb[:], sel_in[:])
            nc.sync.dma_start(w1t0_sb[:], w1t0[:])
            nc.sync.dma_start(w1t1_sb[:], w1t1[:])
            nc.sync.dma_start(wct_sb[:], wct[:])
            nc.scalar.dma_start(xt_sb[:], xt_in[:])
            if has_bias:
                bias_lhsT_sb = constp.tile([2, N_SLOT3], bf16, tag="biasl")
                bias_rhs_sb = constp.tile([2, OUT_DIM], bf16, tag="biasr")
                nc.sync.dma_start(bias_lhsT_sb[:], bias_lhsT[:])
                nc.sync.dma_start(bias_rhs_sb[:], bias_rhs[:])
            xt_v = xt_sb[:].rearrange("p (a b) -> p a b", a=2)

            # ---------------- phase C: gather + PE segsum ----------------
            gi = 0  # gather emission counter; keeps SWDGE sem lane <-> queue fixed
            with tc.tile_pool(name="cps", bufs=3, space="PSUM") as cpp:
                for k, (s0, s1, tok0, ntok) in (
                        [] if "phaseC" in skip
                        else [kv for _ in range(repeat)
                              for kv in enumerate(chunks)]):
                    nsl = s1 - s0
                    ps = cpp.tile([128, 512], f32, tag="ps")
                    nc.tensor.matmul(ps[:, :nsl], zrow[:], zcols[0:1, :nsl],
                                     start=True, stop=False)
                    if ntok and "gather" not in skip:
                        g = gpool.tile([128, TOKCAP3], bf16, tag="g")
                        gv = g[:].rearrange("p (b e) -> p b e", e=ATTR_DIM)
                        # SWDGE ring holds ~1024 descriptors; split the gather
                        for t0 in range(0, ntok, piece):
                            n = min(piece, ntok - t0)
                            nc.gpsimd.dma_gather(
                                gv[:, t0 // 128:(t0 + n) // 128, :],
                                attr_nat[:],
                                widx_sb[:, (tok0 + t0) // 16:(tok0 + t0 + n) // 16],
                                num_idxs=n,
                                num_idxs_reg=n,
                                elem_size=ATTR_DIM,
                                transpose=False,
                                single_packet=single_packet,
                                queue_num=gi % nqueues,
                            )
                            gi += 1
                        c0, ck = chunk_cols[k]
                        if "segmm" not in skip:
                            for (lb, w0, W, cc) in blk_of_chunk[k]:
                                nc.tensor.matmul(
                                    ps[:, w0:w0 + W],
                                    g[:, lb * ATTR_DIM:(lb + 1) * ATTR_DIM],
                                    sel_sb[:, c0 + cc:c0 + cc + W],
                                    start=False, stop=False)
                    nc.tensor.matmul(ps[:, :nsl], zrow[:], zcols[0:1, :nsl],
                                     start=False, stop=True)
                    nc.scalar.copy(sTbf[:, s0:s0 + nsl], ps[:, :nsl])
            if "phaseC" in skip and "phaseD" not in skip:
                nc.vector.memset(sTbf[:], 0.0)

            # ---------------- phase D: dense matmuls + relu + store ------
            sg = store_grp * 128
            with tc.tile_pool(name="eppsum", bufs=4, space="PSUM") as eppp:
                for g0 in ([] if "phaseD" in skip
                           else [kk for _ in range(repeat)
                                 for kk in range(0, N_SLOT3, sg)]):
                    ns = min(sg, N_SLOT3 - g0)
                    ngrp = ns // 128
                    ostage = workp.tile([128, store_grp * OUT_DIM], bf16, tag="ost")
                    for j in range(ngrp):
                        k0 = g0 + j * 128
                        po = eppp.tile([128, OUT_DIM], f32, tag="po")
                        nc.tensor.matmul(po[:], xt_v[:, 0, k0:k0 + 128], w1t0_sb[:],
                                         start=True, stop=False)
                        nc.tensor.matmul(po[:], xt_v[:, 1, k0:k0 + 128], w1t1_sb[:],
                                         start=False, stop=False)
                        nc.tensor.matmul(po[:], sTbf[:, k0:k0 + 128], wct_sb[:],
                                         start=False, stop=not has_bias)
                        if has_bias:
                            nc.tensor.matmul(po[:], bias_lhsT_sb[:, k0:k0 + 128],
                                             bias_rhs_sb[:], start=False, stop=True)
                        nc.scalar.activation(
                            ostage[:, j * OUT_DIM:(j + 1) * OUT_DIM], po[:],
                            mybir.ActivationFunctionType.Relu)
                    nc.sync.dma_start(
                        out[g0:g0 + ns, :].rearrange("(g p) d -> p g d", p=128),
                        ostage[:, :ngrp * OUT_DIM].rearrange(
                            "p (g d) -> p g d", d=OUT_DIM))

    nc.finalize()
    return nc


def kernel3(object_feats, attr_feats, edge_index, edge_weight,
            W_a2o, b_a2o, W_proj, b_proj, W_upd, b_upd):
    structure, plan, in_maps, perms = _prepare3(
        object_feats, attr_feats, edge_index, edge_weight,
        W_a2o, b_a2o, W_proj, b_proj, W_upd, b_upd)
    nc = _compiled_cache.get(structure)
    if nc is None:
        nc = _build3(*plan)
        _compiled_cache[structure] = nc
    res = run_bass_kernel_spmd(nc, in_maps, list(range(NCORES))).results
    out = np.empty((B, N_OBJ, OUT_DIM), dtype=np.float32)
    for c in range(NCORES):
        out[c][perms[c]] = res[c]["out"][:N_OBJ].astype(np.float32)
    return out


# ================================================================ entry point
def kernel(**inputs):
    """Main entry: v3 PE-matmul-segsum pipeline (HW-validated, rel err ~2.3e-3)."""
    return kernel3(**inputs)


def kernel_v1(object_feats, attr_feats, edge_index, edge_weight,
              W_a2o, b_a2o, W_proj, b_proj, W_upd, b_upd):
    structure, plan, in_maps, metas = _prepare(
        object_feats, attr_feats, edge_index, edge_weight,
        W_a2o, b_a2o, W_proj, b_proj, W_upd, b_upd)
    chunks, T, n_slot, has_bias = plan

    nc = _compiled_cache.get(structure)
    if nc is None:
        nc = _build(chunks, T, n_slot, has_bias)
        _compiled_cache[structure] = nc

    res = run_bass_kernel_spmd(nc, in_maps, list(range(NCORES))).results

    out = np.zeros((B, N_OBJ, OUT_DIM), dtype=np.float32)
    for c in range(NCORES):
        perm = metas[c]
        real = perm >= 0
        out[c][perm[real]] = res[c]["out"][real]
    return out

